# revision 1
# baseline (speedup 1.0000x reference)
"""Trainium2 Bass kernel for nn_Net_28544352649361 (segment_reduce).

Reference computation:
    emb_tok[t]   = sum_d word_vectors[tokens[t], d]
    seg_sum[s]   = segment_sum(emb_tok, segment_ids)    (segment_ids sorted)
    lengths[s]   = segment counts
    sv[s]        = seg_sum[s] / max(lengths[s], 1)
    out[s, l]    = sv[s] * sum_d hidden_w[l, d] + hidden_b[l]
(the reference broadcasts the per-sentence scalar over d, so the final Linear
collapses to an outer product against hidden_w's row sums).

Distribution: data-parallel over sentences. Host cuts the token stream at
sentence boundaries s = 2048*c (8 binary searches), pads each shard to a fixed
135168 tokens, and runs one SPMD Bass program on all 8 NeuronCores. Each core:
  - per 128-token column: indirect-DMA row gather wv[tok], DVE reduce over d,
    indirect scatter-ADD (DMA CCE) of [emb, 1.0] into a DRAM accumulator
    indexed by in-shard segment id. 128 rotating accumulators keep concurrent
    scatter-adds race-free (columns 128 apart never share a segment).
  - combine accumulators, sv = sum/max(cnt,1), outer product with the
    broadcast row-sums of hidden_w, add bias, write [2048, 128] rows.
Host concatenates the 8 row blocks.
"""

import sys

sys.path.insert(0, "/opt/trn_rl_repo")

from contextlib import ExitStack

import numpy as np

import concourse.bass as bass
import concourse.tile as tile
from concourse import mybir
from concourse.bass import IndirectOffsetOnAxis
from concourse.vector_clock import ScopedClock

P = 128
F = 1056                 # token columns per core (128*1056 = 135168 slots)
SHARD = P * F
D = 128
NL = 128
NSENT = 16384
NCORES = 8
SENT_PER_CORE = NSENT // NCORES   # 2048
NBAG = 128               # rotating scatter-add accumulators
BAGROWS = 2176           # SENT_PER_CORE + junk rows for padding tokens
PAD_SEG = 2048           # in-shard segment id used for padding tokens (junk row)
UROWS = SENT_PER_CORE // P        # 16

_num_splits = [0]


# ---------------------------------------------------------------------------
# Workarounds for this walrus build (accepts only ONE sync-wait per
# instruction) and Tile's 8-lane DMA-sem round robin.
# ---------------------------------------------------------------------------
def _split_drain_and_barrier(self, tick_clock, wait_clock):
    nc = self.nc
    drain_inst = nc.sync.drain()
    wait_clock.add_sem_waits(
        drain_inst.ins, ScopedClock({None: tick_clock.global_clock})
    )
    mi = drain_inst.ins
    si = mi.sync_info
    if si is not None and si.on_wait is not None and len(si.on_wait) > 1:
        waits = list(si.on_wait)
        si.on_wait = waits[:1]
        for w in waits[1:]:
            extra = nc.sync.drain()
            emi = extra.ins
            esi = emi.sync_info
            if esi is None:
                emi.sync_info = mybir.SyncInfo(on_wait=[w], on_update=[])
            else:
                esi.on_wait = [w]
    nc.all_engine_barrier()
    assert self.sems is not None
    popped = nc._tile_sem_poison_stack.pop()
    assert popped is self._sem_poison
    nc.clear_and_free_semaphores(list(self.sems.allocated().values()))
    nc.all_engine_barrier()


def _apply_patches():
    if getattr(tile, "_segred_patched", False):
        return
    tile.TileContext._drain_and_barrier = _split_drain_and_barrier
    # NOTE: keep all 8 DMA sem lanes — _split_waits() below enforces the
    # 1-sync-wait-per-instruction compiler limit by hoisting surplus waits
    # onto NoOps, and a single lane would make every DMA consumer
    # transitively wait on all earlier DMAs' completions (full round-trip
    # serialization, ~9 us per column).
    tile._segred_patched = True


def _split_waits(nc):
    """Hoist surplus sync-waits onto same-engine NoOps placed just before the
    waiter; the engine sequencer executes them in order."""
    import bass_rust

    for f in nc.m.functions:
        for bb in f.blocks:
            new_list = []
            changed = False
            for inst in bb.instructions:
                si = inst.sync_info
                if si is not None and si.on_wait is not None and len(si.on_wait) > 1:
                    waits = list(si.on_wait)
                    si.on_wait = waits[-1:]
                    for w in waits[:-1]:
                        _num_splits[0] += 1
                        nop = bass_rust.InstNoOp(
                            name=f"WSPLIT-{_num_splits[0]}", ins=[], outs=[]
                        )
                        nop.engine = inst.engine
                        nop.sync_info = mybir.SyncInfo(on_wait=[w], on_update=[])
                        new_list.append(nop)
                    changed = True
                new_list.append(inst)
            if changed:
                bb.instructions = new_list


# ---------------------------------------------------------------------------
# Device program (identical for all cores; per-core data via in_maps)
# ---------------------------------------------------------------------------
def build_program():
    _apply_patches()
    nc = bass.Bass()
    f32 = mybir.dt.float32
    i32 = mybir.dt.int32

    wv = nc.declare_dram_parameter("wv", [100352, D], f32, isOutput=False)
    toks = nc.declare_dram_parameter("toks", [P, F], i32, isOutput=False)
    segs = nc.declare_dram_parameter("segs", [P, F], i32, isOutput=False)
    hwT = nc.declare_dram_parameter("hwT", [D, NL], f32, isOutput=False)
    hb = nc.declare_dram_parameter("hb", [1, NL], f32, isOutput=False)
    out = nc.declare_dram_parameter("out", [SENT_PER_CORE, NL], f32, isOutput=True)

    bags = [nc.dram_tensor(f"bag{r}", [BAGROWS, 2], f32) for r in range(NBAG)]

    with ExitStack() as ctx:
        tc = ctx.enter_context(tile.TileContext(nc))
        const = ctx.enter_context(tc.tile_pool(name="const", bufs=1))
        gp = ctx.enter_context(tc.tile_pool(name="g", bufs=6))
        small = ctx.enter_context(tc.tile_pool(name="small", bufs=2))
        psum = ctx.enter_context(tc.tile_pool(name="ps", bufs=2, space="PSUM"))

        tok_sb = const.tile([P, F], i32)
        seg_sb = const.tile([P, F], i32)
        nc.sync.dma_start(tok_sb[:], toks[:])
        nc.sync.dma_start(seg_sb[:], segs[:])

        # zero-init the accumulators
        z = const.tile([P, 2 * BAGROWS // P], f32)
        nc.vector.memset(z[:], 0.0)
        for r in range(NBAG):
            nc.sync.dma_start(bags[r][:], z[:])

        # main loop: gather rows -> reduce -> scatter-add [emb, 1] into bag.
        # Payload tiles rotate through a pool so the per-column chains pipeline
        # (a single shared payload array would serialize every chain through
        # tile-granular dependency tracking).
        sp = ctx.enter_context(tc.tile_pool(name="s2", bufs=12))
        for k in range(F):
            g = gp.tile([P, D], f32, tag="g")
            nc.gpsimd.indirect_dma_start(
                out=g[:],
                out_offset=None,
                in_=wv[:],
                in_offset=IndirectOffsetOnAxis(ap=tok_sb[:, k : k + 1], axis=0),
            )
            s2 = sp.tile([P, 2], f32, tag="s2")
            nc.vector.memset(s2[:, 1:2], 1.0)
            nc.vector.tensor_reduce(
                s2[:, 0:1],
                g[:],
                axis=mybir.AxisListType.X,
                op=mybir.AluOpType.add,
            )
            nc.gpsimd.indirect_dma_start(
                out=bags[k % NBAG][:],
                out_offset=IndirectOffsetOnAxis(ap=seg_sb[:, k : k + 1], axis=0),
                in_=s2[:],
                in_offset=None,
                compute_op=mybir.AluOpType.add,
            )

        # combine the 128 accumulators: acc[p, u, c] = sum_r bag_r[u*128+p, c]
        acc = const.tile([P, UROWS, 2], f32)
        nc.vector.memset(acc[:], 0.0)
        for r in range(NBAG):
            w = gp.tile([P, UROWS, 2], f32, tag="w")
            nc.sync.dma_start(
                w[:],
                bags[r][:].rearrange("(u p) c -> p u c", p=P)[0:P, 0:UROWS, :],
            )
            nc.vector.tensor_tensor(
                out=acc[:], in0=acc[:], in1=w[:], op=mybir.AluOpType.add
            )

        # sv = seg_sum / max(len, 1)
        lens = small.tile([P, UROWS], f32)
        nc.vector.tensor_scalar_max(lens[:], acc[:, :, 1], 1.0)
        rec = small.tile([P, UROWS], f32)
        nc.vector.reciprocal(rec[:], lens[:])
        sv = small.tile([P, UROWS], f32)
        nc.vector.tensor_tensor(
            out=sv[:], in0=acc[:, :, 0], in1=rec[:], op=mybir.AluOpType.mult
        )

        # broadcast row-sums of hidden_w and the bias to all partitions
        hwT_sb = const.tile([D, NL], f32)
        nc.sync.dma_start(hwT_sb[:], hwT[:])
        hb_sb = const.tile([1, NL], f32)
        nc.sync.dma_start(hb_sb[:], hb[:])
        ones_p = const.tile([P, 1], f32)
        nc.vector.memset(ones_p[:], 1.0)
        ones_1 = const.tile([1, P], f32)
        nc.vector.memset(ones_1[:], 1.0)

        wrow_ps = psum.tile([1, NL], f32, tag="ps1")
        nc.tensor.matmul(wrow_ps[:], ones_p[:], hwT_sb[:], start=True, stop=True)
        wrow = small.tile([1, NL], f32)
        nc.scalar.copy(wrow[:], wrow_ps[:])

        wb_ps = psum.tile([P, NL], f32, tag="ps2")
        nc.tensor.matmul(wb_ps[:], ones_1[:], wrow[:], start=True, stop=True)
        w_b = const.tile([P, NL], f32)
        nc.scalar.copy(w_b[:], wb_ps[:])

        bb_ps = psum.tile([P, NL], f32, tag="ps2")
        nc.tensor.matmul(bb_ps[:], ones_1[:], hb_sb[:], start=True, stop=True)
        b_b = const.tile([P, NL], f32)
        nc.scalar.copy(b_b[:], bb_ps[:])

        # out[u*128+p, l] = sv[p, u] * w_b[p, l] + b_b[p, l]
        out_sb = const.tile([P, UROWS, NL], f32)
        for u in range(UROWS):
            nc.vector.tensor_scalar(
                out=out_sb[:, u, :],
                in0=w_b[:],
                scalar1=sv[:, u : u + 1],
                scalar2=None,
                op0=mybir.AluOpType.mult,
            )
            nc.vector.tensor_tensor(
                out=out_sb[:, u, :],
                in0=out_sb[:, u, :],
                in1=b_b[:],
                op=mybir.AluOpType.add,
            )

        nc.sync.dma_start(
            out[:].rearrange("(u p) l -> p u l", p=P)[0:P, 0:UROWS, :], out_sb[:]
        )

    _split_waits(nc)
    return nc


_PROGRAM = None


def _get_program():
    global _PROGRAM
    if _PROGRAM is None:
        _PROGRAM = build_program()
    return _PROGRAM


def kernel(tokens, segment_ids, word_vectors, hidden_w, hidden_b):
    from concourse.bass_utils import run_bass_kernel_spmd

    tokens = np.asarray(tokens)
    segment_ids = np.asarray(segment_ids)
    word_vectors = np.asarray(word_vectors, dtype=np.float32)
    hidden_w = np.asarray(hidden_w, dtype=np.float32)
    hidden_b = np.asarray(hidden_b, dtype=np.float32)

    # replicate-pad the embedding table to the declared 100352 rows
    wv_pad = np.zeros((100352, D), dtype=np.float32)
    wv_pad[: word_vectors.shape[0]] = word_vectors
    hwT = np.ascontiguousarray(hidden_w.T)
    hb = hidden_b.reshape(1, NL)

    # sentence-aligned cuts: core c owns sentences [2048c, 2048(c+1))
    cuts = np.searchsorted(segment_ids, np.arange(NCORES + 1) * SENT_PER_CORE)
    in_maps = []
    for c in range(NCORES):
        lo, hi = int(cuts[c]), int(cuts[c + 1])
        n = hi - lo
        assert n <= SHARD, f"shard {c} has {n} tokens > {SHARD}"
        tk = np.zeros(SHARD, dtype=np.int32)
        sg = np.full(SHARD, PAD_SEG, dtype=np.int32)
        tk[:n] = tokens[lo:hi]
        sg[:n] = segment_ids[lo:hi] - c * SENT_PER_CORE
        in_maps.append(
            {
                "wv": wv_pad,
                "toks": tk.reshape(P, F),
                "segs": sg.reshape(P, F),
                "hwT": hwT,
                "hb": hb,
            }
        )

    nc = _get_program()
    res = run_bass_kernel_spmd(nc, in_maps, list(range(NCORES)))
    return np.concatenate([res.results[c]["out"] for c in range(NCORES)], axis=0)



# revision 2
# speedup vs baseline: 1.1156x; 1.1156x over previous
"""Trainium2 Bass kernel for nn_Net_28544352649361 (segment_reduce), v2.

Per core (2048 sentences, <=133120 tokens laid out partition-major:
token q -> (p = q // 1040, j = q % 1040)):
  1. rowsum[v] = sum_d wv[v, d]: reduce own 12544-row vocab shard (6.4MB
     dense read + DVE reduce), AllGather -> full [100352] table in DRAM.
  2. Lane tables: SBUF data[p, e] = rowsum[6272*(p%16) + e] via one
     broadcast-AP DMA; each 16-partition GPSIMD group holds the whole vocab.
  3. Gather: 8x indirect_copy fetches entry e(v)=v%6272 on all 16 lanes of
     the owning group; host bf16... f32 mask keeps lane v//6272; bf16
     mask-product collapses via 0/1-stationary matmuls that route slot
     (g, r*1040+j) -> psum_v[8r+g, j] = V in token order.
  4. cum: DVE tensor_tensor_scan along free dim + cross-partition exclusive
     prefix (tri matmul) + per-partition offset add; cum -> DRAM (flat = q).
  5. Segment sums: 16 indirect gathers of cum at segment-end positions;
     predecessor ends via shift matmuls; sv = (G1 - G2) / max(len, 1).
  6. out[s, l] = sv[s] * rowsum(hidden_w)[l] + hb[l].
Host prep is integer-only: shard cuts, slot/lane indices, end offsets, lens.
"""

import sys

sys.path.insert(0, "/opt/trn_rl_repo")

from contextlib import ExitStack

import dataclasses
import numpy as np
import ml_dtypes

import concourse.bass as bass
import concourse.tile as tile
from concourse import mybir
from concourse.bass import IndirectOffsetOnAxis
from concourse.vector_clock import ScopedClock

P = 128
J = 1040                  # free-dim tokens per partition; slots = 128*J
SLOTS = P * J             # 133120 >= max shard (131371)
IC = 16 * J               # indirect_copy output columns (8 groups x 2 rounds)
NCHUNK = 8
CCOL = IC // NCHUNK       # 2080 ic-columns per chunk (= 2 rounds of J)
D = 128
NL = 128
NSENT = 16384
NCORES = 8
SENT_PER_CORE = NSENT // NCORES   # 2048
K16 = SENT_PER_CORE // P          # 16
VOC = 100000
VOC_PAD = 100352
VSHARD = VOC_PAD // NCORES        # 12544
LANE = VOC_PAD // 16              # 6272
U = VSHARD // P                   # 98 rows per partition in rowsum build

_num_splits = [0]


# ---------------------------------------------------------------------------
# Workarounds for this walrus build (accepts only ONE sync-wait per
# instruction) and Tile's drain path.
# ---------------------------------------------------------------------------
def _split_drain_and_barrier(self, tick_clock, wait_clock):
    nc = self.nc
    drain_inst = nc.sync.drain()
    wait_clock.add_sem_waits(
        drain_inst.ins, ScopedClock({None: tick_clock.global_clock})
    )
    mi = drain_inst.ins
    si = mi.sync_info
    if si is not None and si.on_wait is not None and len(si.on_wait) > 1:
        waits = list(si.on_wait)
        si.on_wait = waits[:1]
        for w in waits[1:]:
            extra = nc.sync.drain()
            emi = extra.ins
            esi = emi.sync_info
            if esi is None:
                emi.sync_info = mybir.SyncInfo(on_wait=[w], on_update=[])
            else:
                esi.on_wait = [w]
    nc.all_engine_barrier()
    assert self.sems is not None
    popped = nc._tile_sem_poison_stack.pop()
    assert popped is self._sem_poison
    nc.clear_and_free_semaphores(list(self.sems.allocated().values()))
    nc.all_engine_barrier()


def _apply_patches():
    if getattr(tile, "_segred_patched", False):
        return
    tile.TileContext._drain_and_barrier = _split_drain_and_barrier
    tile._segred_patched = True


def _split_waits(nc):
    """Hoist surplus sync-waits onto same-engine NoOps placed just before the
    waiter; the engine sequencer executes them in order."""
    import bass_rust

    for f in nc.m.functions:
        for bb in f.blocks:
            new_list = []
            changed = False
            for inst in bb.instructions:
                si = inst.sync_info
                if si is not None and si.on_wait is not None and len(si.on_wait) > 1:
                    waits = list(si.on_wait)
                    si.on_wait = waits[-1:]
                    for w in waits[:-1]:
                        _num_splits[0] += 1
                        nop = bass_rust.InstNoOp(
                            name=f"WSPLIT-{_num_splits[0]}", ins=[], outs=[]
                        )
                        nop.engine = inst.engine
                        nop.sync_info = mybir.SyncInfo(on_wait=[w], on_update=[])
                        new_list.append(nop)
                    changed = True
                new_list.append(inst)
            if changed:
                bb.instructions = new_list


# ---------------------------------------------------------------------------
# Device program (identical for all cores; per-core data via in_maps)
# ---------------------------------------------------------------------------
def build_program():
    _apply_patches()
    nc = bass.Bass(num_devices=NCORES)
    f32 = mybir.dt.float32
    i32 = mybir.dt.int32
    u16 = mybir.dt.uint16
    bf16 = mybir.dt.bfloat16

    wvs = nc.declare_dram_parameter("wvs", [VSHARD, D], f32, isOutput=False)
    icidx = nc.declare_dram_parameter("icidx", [P, 1040], u16, isOutput=False)
    lmask = nc.declare_dram_parameter("lmask", [P, IC], f32, isOutput=False)
    endoff = nc.declare_dram_parameter("endoff", [P, K16], i32, isOutput=False)
    lens = nc.declare_dram_parameter("lens", [P, K16], f32, isOutput=False)
    swide = nc.declare_dram_parameter("swide", [P, 248], bf16, isOutput=False)
    triex = nc.declare_dram_parameter("triex", [P, P], f32, isOutput=False)
    subm = nc.declare_dram_parameter("subm", [P, P], f32, isOutput=False)
    cornm = nc.declare_dram_parameter("cornm", [P, P], f32, isOutput=False)
    hwT = nc.declare_dram_parameter("hwT", [D, NL], f32, isOutput=False)
    hb = nc.declare_dram_parameter("hb", [1, NL], f32, isOutput=False)
    out = nc.declare_dram_parameter("out", [SENT_PER_CORE, NL], f32, isOutput=True)

    rs_shard = nc.dram_tensor("rs_shard", [1, VSHARD], f32)
    rs_full = nc.dram_tensor("rs_full", [NCORES, VSHARD], f32)
    cum_dram = nc.dram_tensor("cum_dram", [SLOTS, 1], f32)

    with ExitStack() as ctx:
        tc = ctx.enter_context(tile.TileContext(nc))
        const = ctx.enter_context(tc.tile_pool(name="const", bufs=1))
        gp = ctx.enter_context(tc.tile_pool(name="g", bufs=2))
        mp = ctx.enter_context(tc.tile_pool(name="m", bufs=2))
        small = ctx.enter_context(tc.tile_pool(name="small", bufs=3))
        ge = ctx.enter_context(tc.tile_pool(name="ge", bufs=16))
        pbig = ctx.enter_context(tc.tile_pool(name="pb", bufs=1, space="PSUM"))
        psm = ctx.enter_context(tc.tile_pool(name="psm", bufs=1, space="PSUM"))

        # --- constants / small inputs ---------------------------------------
        icidx_sb = const.tile([P, 1040], u16)
        nc.sync.dma_start(icidx_sb[:], icidx[:])
        lmask_sb = const.tile([P, IC], f32)
        nc.sync.dma_start(lmask_sb[:], lmask[:])
        endoff_sb = const.tile([P, K16], i32)
        nc.sync.dma_start(endoff_sb[:], endoff[:])
        lens_sb = const.tile([P, K16], f32)
        nc.sync.dma_start(lens_sb[:], lens[:])
        swide_sb = const.tile([P, 248], bf16)
        nc.sync.dma_start(swide_sb[:], swide[:])
        triex_sb = const.tile([P, P], f32)
        nc.sync.dma_start(triex_sb[:], triex[:])
        subm_sb = const.tile([P, P], f32)
        nc.sync.dma_start(subm_sb[:], subm[:])
        cornm_sb = const.tile([P, P], f32)
        nc.sync.dma_start(cornm_sb[:], cornm[:])
        hwT_sb = const.tile([D, NL], f32)
        nc.sync.dma_start(hwT_sb[:], hwT[:])
        hb_sb = const.tile([1, NL], f32)
        nc.sync.dma_start(hb_sb[:], hb[:])
        ones_p = const.tile([P, 1], f32)
        nc.vector.memset(ones_p[:], 1.0)
        ones_1 = const.tile([1, P], f32)
        nc.vector.memset(ones_1[:], 1.0)
        zeros_j = const.tile([P, J], f32)
        nc.vector.memset(zeros_j[:], 0.0)

        # --- w_b / b_b broadcast --------------------------------------------
        wrow_ps = psm.tile([1, NL], f32, tag="ps1")
        nc.tensor.matmul(wrow_ps[:], ones_p[:], hwT_sb[:], start=True, stop=True)
        wrow = small.tile([1, NL], f32)
        nc.scalar.copy(wrow[:], wrow_ps[:])
        wb_ps = psm.tile([P, NL], f32, tag="ps2")
        nc.tensor.matmul(wb_ps[:], ones_1[:], wrow[:], start=True, stop=True)
        w_b = const.tile([P, NL], f32)
        nc.scalar.copy(w_b[:], wb_ps[:])
        bb_ps = psm.tile([P, NL], f32, tag="ps2")
        nc.tensor.matmul(bb_ps[:], ones_1[:], hb_sb[:], start=True, stop=True)
        b_b = const.tile([P, NL], f32)
        nc.scalar.copy(b_b[:], bb_ps[:])

        # --- rowsum shard: load [128, 49*128] x2, reduce over d -------------
        rs_sb = const.tile([P, U], f32)
        wv_view = wvs[:].rearrange("(p u) d -> p u d", p=P)
        for h in range(2):
            u0, u1 = 49 * h, 49 * (h + 1)
            wvt = gp.tile([P, 49, D], f32, tag="wv")
            nc.sync.dma_start(wvt[:], wv_view[0:P, u0:u1, :])
            nc.vector.tensor_reduce(
                rs_sb[:, u0:u1],
                wvt[:],
                axis=mybir.AxisListType.X,
                op=mybir.AluOpType.add,
            )
        nc.sync.dma_start(
            rs_shard[:].rearrange("one (p u) -> p (one u)", p=P), rs_sb[:]
        )
        nc.gpsimd.collective_compute(
            "AllGather",
            mybir.AluOpType.bypass,
            replica_groups=[list(range(NCORES))],
            ins=[rs_shard[:].opt()],
            outs=[rs_full[:].opt()],
        )

        # --- lane tables: data[p, e] = rowsum[6272*(p%16) + e] --------------
        data_sb = const.tile([P, LANE], f32)
        lanes16 = rs_full[:].rearrange("c (l e) -> (c l) e", l=2)   # [16, 6272]
        for g in range(8):
            eng = nc.sync if g % 2 == 0 else nc.gpsimd
            eng.dma_start(data_sb[16 * g:16 * g + 16, :], lanes16[:, :])

        # --- gather + mask + collapse into psum_v[128, J] -------------------
        # indirect_copy caps at 1024 valid indices -> 1024 + 16 split per round
        psum_v = pbig.tile([P, J], f32, tag="pv")
        for r in range(16):
            ic_a = gp.tile([P, 1024], f32, tag="ica")
            nc.gpsimd.indirect_copy(
                out=ic_a[:],
                data=data_sb[:],
                idxs=icidx_sb[:, 64 * r:64 * r + 64],
                i_know_ap_gather_is_preferred=True,
            )
            mk_t = mp.tile([P, 1024], bf16, tag="mk")
            nc.vector.tensor_tensor(
                out=mk_t[:],
                in0=ic_a[:],
                in1=lmask_sb[:, 1024 * r:1024 * (r + 1)],
                op=mybir.AluOpType.mult,
            )
            stat = swide_sb[:, 120 - 8 * r:248 - 8 * r]
            for j0, j1 in ((0, 512), (512, 1024)):
                nc.tensor.matmul(
                    psum_v[:, j0:j1],
                    stat,
                    mk_t[:, j0:j1],
                    start=(r == 0),
                    stop=(r == 15),
                )
        # tail: slots j in [1024, 1040) for all 16 rounds in one gather
        ic_c = gp.tile([P, 256], f32, tag="icc")
        nc.gpsimd.indirect_copy(
            out=ic_c[:],
            data=data_sb[:],
            idxs=icidx_sb[:, 1024:1040],
            i_know_ap_gather_is_preferred=True,
        )
        mk_c = mp.tile([P, 256], bf16, tag="mkc")
        nc.vector.tensor_tensor(
            out=mk_c[:],
            in0=ic_c[:],
            in1=lmask_sb[:, 16384:16640],
            op=mybir.AluOpType.mult,
        )
        for r in range(16):
            nc.tensor.matmul(
                psum_v[:, 1024:J],
                swide_sb[:, 120 - 8 * r:248 - 8 * r],
                mk_c[:, 16 * r:16 * (r + 1)],
                start=(r == 0),
                stop=(r == 15),
            )

        # --- cum = scan(V) + cross-partition offsets ------------------------
        cum_part = const.tile([P, J], f32)
        nc.vector.tensor_tensor_scan(
            out=cum_part[:],
            data0=psum_v[:],
            data1=zeros_j[:],
            initial=0.0,
            op0=mybir.AluOpType.add,
            op1=mybir.AluOpType.add,
        )
        pt_ps = psm.tile([P, 1], f32, tag="pt")
        nc.tensor.matmul(
            pt_ps[:], triex_sb[:], cum_part[:, J - 1:J], start=True, stop=True
        )
        cum_sb = const.tile([P, J], f32)
        nc.vector.tensor_scalar(
            out=cum_sb[:], in0=cum_part[:], scalar1=pt_ps[:], scalar2=None,
            op0=mybir.AluOpType.add,
        )
        nc.sync.dma_start(
            cum_dram[:].rearrange("(p j) one -> p (j one)", p=P), cum_sb[:]
        )

        # --- segment ends ----------------------------------------------------
        g1s = const.tile([P, K16 + 1], f32)
        nc.vector.memset(g1s[:, 0:1], 0.0)
        for k in range(K16):
            gt = ge.tile([P, 1], f32, tag="gt")
            nc.gpsimd.indirect_dma_start(
                out=gt[:],
                out_offset=None,
                in_=cum_dram[:],
                in_offset=IndirectOffsetOnAxis(ap=endoff_sb[:, k:k + 1], axis=0),
            )
            nc.scalar.copy(g1s[:, 1 + k:2 + k], gt[:])
        # G2 = shift(G1): sub-diagonal + corner
        g2_ps = psm.tile([P, K16], f32, tag="g2")
        nc.tensor.matmul(
            g2_ps[:], subm_sb[:], g1s[:, 1:K16 + 1], start=True, stop=False
        )
        nc.tensor.matmul(
            g2_ps[:], cornm_sb[:], g1s[:, 0:K16], start=False, stop=True
        )

        segsum = small.tile([P, K16], f32)
        nc.vector.tensor_tensor(
            out=segsum[:], in0=g1s[:, 1:K16 + 1], in1=g2_ps[:],
            op=mybir.AluOpType.subtract,
        )
        lm = small.tile([P, K16], f32)
        nc.vector.tensor_scalar_max(lm[:], lens_sb[:], 1.0)
        rec = small.tile([P, K16], f32)
        nc.vector.reciprocal(rec[:], lm[:])
        sv = small.tile([P, K16], f32)
        nc.vector.tensor_tensor(
            out=sv[:], in0=segsum[:], in1=rec[:], op=mybir.AluOpType.mult
        )

        # --- out[128k+p, l] = sv[p, k] * w_b[p, l] + b_b[p, l] ---------------
        out_sb = const.tile([P, K16, NL], f32)
        for k in range(K16):
            nc.vector.tensor_scalar(
                out=out_sb[:, k, :], in0=w_b[:],
                scalar1=sv[:, k:k + 1], scalar2=None,
                op0=mybir.AluOpType.mult,
            )
            nc.vector.tensor_tensor(
                out=out_sb[:, k, :], in0=out_sb[:, k, :], in1=b_b[:],
                op=mybir.AluOpType.add,
            )
        nc.sync.dma_start(out[:].rearrange("(k p) l -> p k l", p=P), out_sb[:])

    _split_waits(nc)
    return nc


_PROGRAM = None


def _get_program():
    global _PROGRAM
    if _PROGRAM is None:
        _PROGRAM = build_program()
    return _PROGRAM


def _host_prep(tokens, segment_ids, word_vectors, hidden_w, hidden_b):
    """Integer-only preprocessing: shard + slot/lane/end index tensors."""
    tokens = np.asarray(tokens)
    segment_ids = np.asarray(segment_ids)
    wv = np.asarray(word_vectors, dtype=np.float32)
    hw = np.asarray(hidden_w, dtype=np.float32)
    hbv = np.asarray(hidden_b, dtype=np.float32)

    wv_pad = np.zeros((VOC_PAD, D), dtype=np.float32)
    wv_pad[:VOC] = wv
    hwT = np.ascontiguousarray(hw.T)
    hbr = hbv.reshape(1, NL)

    triex = np.triu(np.ones((P, P), dtype=np.float32), 1)       # pi < po
    subm = np.zeros((P, P), dtype=np.float32)
    subm[np.arange(P - 1), np.arange(1, P)] = 1.0               # po = pi+1
    cornm = np.zeros((P, P), dtype=np.float32)
    cornm[P - 1, 0] = 1.0                                       # po=0 <- pi=127
    swide = np.zeros((P, 248), dtype=np.float32)
    swide[np.arange(P), 120 + np.arange(P) // 16] = 1.0
    swide = swide.astype(ml_dtypes.bfloat16)

    cuts = np.searchsorted(segment_ids, np.arange(NCORES + 1) * SENT_PER_CORE)
    in_maps = []
    for c in range(NCORES):
        lo, hi = int(cuts[c]), int(cuts[c + 1])
        n = hi - lo
        assert n <= SLOTS, f"shard {c}: {n} > {SLOTS}"
        toks = tokens[lo:hi].astype(np.int64)
        segs = (segment_ids[lo:hi] - c * SENT_PER_CORE).astype(np.int64)

        q = np.arange(n)
        p = q // J                 # partition-major token layout
        j = q % J
        g = p % 8
        r = p // 8
        i = r * J + j
        lane = toks // LANE
        e = toks % LANE

        icidx_a = np.zeros((P, 1040), dtype=np.uint16)
        lo_j = j < 1024
        icidx_a[16 * g[lo_j] + j[lo_j] % 16,
                64 * r[lo_j] + j[lo_j] // 16] = e[lo_j].astype(np.uint16)
        hi_j = ~lo_j
        it = r[hi_j] * 16 + (j[hi_j] - 1024)       # tail slot index [0, 256)
        icidx_a[16 * g[hi_j] + it % 16, 1024 + it // 16] = e[hi_j].astype(np.uint16)
        lmask_a = np.zeros((P, IC), dtype=np.float32)
        im = np.where(j < 1024, 1024 * r + j, 16384 + r * 16 + (j - 1024))
        lmask_a[16 * g + lane, im] = 1.0

        ends_q = np.searchsorted(segs, np.arange(SENT_PER_CORE), side="right") - 1
        lens_a = np.bincount(segs, minlength=SENT_PER_CORE).astype(np.float32)
        assert lens_a.min() >= 1, f"shard {c} has empty segments"
        endoff_a = ends_q.reshape(K16, P).T.astype(np.int32).copy()
        lens_t = lens_a.reshape(K16, P).T.astype(np.float32).copy()

        in_maps.append(
            {
                "wvs": wv_pad[c * VSHARD:(c + 1) * VSHARD],
                "icidx": icidx_a,
                "lmask": lmask_a,
                "endoff": endoff_a,
                "lens": lens_t,
                "swide": swide,
                "triex": triex,
                "subm": subm,
                "cornm": cornm,
                "hwT": hwT,
                "hb": hbr,
            }
        )
    return in_maps


def kernel(tokens, segment_ids, word_vectors, hidden_w, hidden_b):
    from concourse.bass_utils import run_bass_kernel_spmd

    in_maps = _host_prep(tokens, segment_ids, word_vectors, hidden_w, hidden_b)
    nc = _get_program()
    res = run_bass_kernel_spmd(nc, in_maps, list(range(NCORES)))
    return np.concatenate([res.results[c]["out"] for c in range(NCORES)], axis=0)


# revision 3
# speedup vs baseline: 1.1464x; 1.0276x over previous
"""Trainium2 Bass kernel for nn_Net_28544352649361 (segment_reduce), v2.

Per core (2048 sentences, <=133120 tokens laid out partition-major:
token q -> (p = q // 1040, j = q % 1040)):
  1. rowsum[v] = sum_d wv[v, d]: reduce own 12544-row vocab shard (6.4MB
     dense read + DVE reduce), AllGather -> full [100352] table in DRAM.
  2. Lane tables: SBUF data[p, e] = rowsum[6272*(p%16) + e] via one
     broadcast-AP DMA; each 16-partition GPSIMD group holds the whole vocab.
  3. Gather: 8x indirect_copy fetches entry e(v)=v%6272 on all 16 lanes of
     the owning group; host bf16... f32 mask keeps lane v//6272; bf16
     mask-product collapses via 0/1-stationary matmuls that route slot
     (g, r*1040+j) -> psum_v[8r+g, j] = V in token order.
  4. cum: DVE tensor_tensor_scan along free dim + cross-partition exclusive
     prefix (tri matmul) + per-partition offset add; cum -> DRAM (flat = q).
  5. Segment sums: 16 indirect gathers of cum at segment-end positions;
     predecessor ends via shift matmuls; sv = (G1 - G2) / max(len, 1).
  6. out[s, l] = sv[s] * rowsum(hidden_w)[l] + hb[l].
Host prep is integer-only: shard cuts, slot/lane indices, end offsets, lens.
"""

import sys

sys.path.insert(0, "/opt/trn_rl_repo")

from contextlib import ExitStack

import dataclasses
import numpy as np
import ml_dtypes

import concourse.bass as bass
import concourse.tile as tile
from concourse import mybir
from concourse.bass import IndirectOffsetOnAxis
from concourse.vector_clock import ScopedClock

P = 128
J = 1040                  # free-dim tokens per partition; slots = 128*J
SLOTS = P * J             # 133120 >= max shard (131371)
IC = 16 * J               # indirect_copy output columns (8 groups x 2 rounds)
NCHUNK = 8
CCOL = IC // NCHUNK       # 2080 ic-columns per chunk (= 2 rounds of J)
D = 128
NL = 128
NSENT = 16384
NCORES = 8
SENT_PER_CORE = NSENT // NCORES   # 2048
K16 = SENT_PER_CORE // P          # 16
VOC = 100000
VOC_PAD = 100352
VSHARD = VOC_PAD // NCORES        # 12544
LANE = VOC_PAD // 16              # 6272
U = VSHARD // P                   # 98 rows per partition in rowsum build

_num_splits = [0]


# ---------------------------------------------------------------------------
# Workarounds for this walrus build (accepts only ONE sync-wait per
# instruction) and Tile's drain path.
# ---------------------------------------------------------------------------
def _split_drain_and_barrier(self, tick_clock, wait_clock):
    nc = self.nc
    drain_inst = nc.sync.drain()
    wait_clock.add_sem_waits(
        drain_inst.ins, ScopedClock({None: tick_clock.global_clock})
    )
    mi = drain_inst.ins
    si = mi.sync_info
    if si is not None and si.on_wait is not None and len(si.on_wait) > 1:
        waits = list(si.on_wait)
        si.on_wait = waits[:1]
        for w in waits[1:]:
            extra = nc.sync.drain()
            emi = extra.ins
            esi = emi.sync_info
            if esi is None:
                emi.sync_info = mybir.SyncInfo(on_wait=[w], on_update=[])
            else:
                esi.on_wait = [w]
    nc.all_engine_barrier()
    assert self.sems is not None
    popped = nc._tile_sem_poison_stack.pop()
    assert popped is self._sem_poison
    nc.clear_and_free_semaphores(list(self.sems.allocated().values()))
    nc.all_engine_barrier()


def _apply_patches():
    if getattr(tile, "_segred_patched", False):
        return
    tile.TileContext._drain_and_barrier = _split_drain_and_barrier
    tile._segred_patched = True


def _split_waits(nc):
    """Hoist surplus sync-waits onto same-engine NoOps placed just before the
    waiter; the engine sequencer executes them in order."""
    import bass_rust

    for f in nc.m.functions:
        for bb in f.blocks:
            new_list = []
            changed = False
            for inst in bb.instructions:
                si = inst.sync_info
                if si is not None and si.on_wait is not None and len(si.on_wait) > 1:
                    waits = list(si.on_wait)
                    si.on_wait = waits[-1:]
                    for w in waits[:-1]:
                        _num_splits[0] += 1
                        nop = bass_rust.InstNoOp(
                            name=f"WSPLIT-{_num_splits[0]}", ins=[], outs=[]
                        )
                        nop.engine = inst.engine
                        nop.sync_info = mybir.SyncInfo(on_wait=[w], on_update=[])
                        new_list.append(nop)
                    changed = True
                new_list.append(inst)
            if changed:
                bb.instructions = new_list


# ---------------------------------------------------------------------------
# Device program (identical for all cores; per-core data via in_maps)
# ---------------------------------------------------------------------------
def build_program():
    _apply_patches()
    nc = bass.Bass(num_devices=NCORES)
    f32 = mybir.dt.float32
    i32 = mybir.dt.int32
    u16 = mybir.dt.uint16
    bf16 = mybir.dt.bfloat16

    wvs = nc.declare_dram_parameter("wvs", [VSHARD, D], f32, isOutput=False)
    icidx = nc.declare_dram_parameter("icidx", [P, 1040], u16, isOutput=False)
    lmask = nc.declare_dram_parameter("lmask", [P, IC], f32, isOutput=False)
    endoff = nc.declare_dram_parameter("endoff", [P, K16], i32, isOutput=False)
    lens = nc.declare_dram_parameter("lens", [P, K16], f32, isOutput=False)
    swide = nc.declare_dram_parameter("swide", [P, 248], bf16, isOutput=False)
    triex = nc.declare_dram_parameter("triex", [P, P], f32, isOutput=False)
    subm = nc.declare_dram_parameter("subm", [P, P], f32, isOutput=False)
    cornm = nc.declare_dram_parameter("cornm", [P, P], f32, isOutput=False)
    hwT = nc.declare_dram_parameter("hwT", [D, NL], f32, isOutput=False)
    hb = nc.declare_dram_parameter("hb", [1, NL], f32, isOutput=False)
    out = nc.declare_dram_parameter("out", [SENT_PER_CORE, NL], f32, isOutput=True)

    rs_shard = nc.dram_tensor("rs_shard", [1, VSHARD], f32)
    rs_full = nc.dram_tensor("rs_full", [NCORES, VSHARD], f32)
    cum_dram = nc.dram_tensor("cum_dram", [SLOTS, 1], f32)

    with ExitStack() as ctx:
        tc = ctx.enter_context(tile.TileContext(nc))
        const = ctx.enter_context(tc.tile_pool(name="const", bufs=1))
        gp = ctx.enter_context(tc.tile_pool(name="g", bufs=2))
        mp = ctx.enter_context(tc.tile_pool(name="m", bufs=2))
        small = ctx.enter_context(tc.tile_pool(name="small", bufs=3))
        ge = ctx.enter_context(tc.tile_pool(name="ge", bufs=16))
        pbig = ctx.enter_context(tc.tile_pool(name="pb", bufs=1, space="PSUM"))
        psm = ctx.enter_context(tc.tile_pool(name="psm", bufs=1, space="PSUM"))

        # --- rowsum shard first: its loads/reduce/AllGather gate everything -
        rs_sb = const.tile([P, U], f32)
        wv_view = wvs[:].rearrange("(p u) d -> p u d", p=P)
        bounds = [(0, 24), (24, 48), (48, 72), (72, U)]
        for u0, u1 in bounds:
            wvt = gp.tile([P, u1 - u0, D], f32, tag="wv")
            nc.sync.dma_start(wvt[:], wv_view[0:P, u0:u1, :])
            nc.vector.tensor_reduce(
                rs_sb[:, u0:u1],
                wvt[:],
                axis=mybir.AxisListType.X,
                op=mybir.AluOpType.add,
            )
        nc.sync.dma_start(
            rs_shard[:].rearrange("one (p u) -> p (one u)", p=P), rs_sb[:]
        )
        nc.gpsimd.collective_compute(
            "AllGather",
            mybir.AluOpType.bypass,
            replica_groups=[list(range(NCORES))],
            ins=[rs_shard[:].opt()],
            outs=[rs_full[:].opt()],
        )

        # --- constants / small inputs (overlap the AllGather window) --------
        icidx_sb = const.tile([P, 1040], u16)
        nc.sync.dma_start(icidx_sb[:], icidx[:])
        lmask_sb = const.tile([P, IC], f32)
        for q0 in range(0, IC, IC // 4):
            nc.sync.dma_start(
                lmask_sb[:, q0:q0 + IC // 4], lmask[:, q0:q0 + IC // 4]
            )
        endoff_sb = const.tile([P, K16], i32)
        nc.sync.dma_start(endoff_sb[:], endoff[:])
        lens_sb = const.tile([P, K16], f32)
        nc.sync.dma_start(lens_sb[:], lens[:])
        swide_sb = const.tile([P, 248], bf16)
        nc.sync.dma_start(swide_sb[:], swide[:])
        triex_sb = const.tile([P, P], f32)
        nc.sync.dma_start(triex_sb[:], triex[:])
        subm_sb = const.tile([P, P], f32)
        nc.sync.dma_start(subm_sb[:], subm[:])
        cornm_sb = const.tile([P, P], f32)
        nc.sync.dma_start(cornm_sb[:], cornm[:])
        hwT_sb = const.tile([D, NL], f32)
        nc.sync.dma_start(hwT_sb[:], hwT[:])
        hb_sb = const.tile([1, NL], f32)
        nc.sync.dma_start(hb_sb[:], hb[:])
        ones_p = const.tile([P, 1], f32)
        nc.vector.memset(ones_p[:], 1.0)
        ones_1 = const.tile([1, P], f32)
        nc.vector.memset(ones_1[:], 1.0)
        zeros_j = const.tile([P, J], f32)
        nc.vector.memset(zeros_j[:], 0.0)

        # --- w_b / b_b broadcast --------------------------------------------
        wrow_ps = psm.tile([1, NL], f32, tag="ps1")
        nc.tensor.matmul(wrow_ps[:], ones_p[:], hwT_sb[:], start=True, stop=True)
        wrow = small.tile([1, NL], f32)
        nc.scalar.copy(wrow[:], wrow_ps[:])
        wb_ps = psm.tile([P, NL], f32, tag="ps2")
        nc.tensor.matmul(wb_ps[:], ones_1[:], wrow[:], start=True, stop=True)
        w_b = const.tile([P, NL], f32)
        nc.scalar.copy(w_b[:], wb_ps[:])
        bb_ps = psm.tile([P, NL], f32, tag="ps2")
        nc.tensor.matmul(bb_ps[:], ones_1[:], hb_sb[:], start=True, stop=True)
        b_b = const.tile([P, NL], f32)
        nc.scalar.copy(b_b[:], bb_ps[:])

        # --- lane tables: data[p, e] = rowsum[6272*(p%16) + e] --------------
        data_sb = const.tile([P, LANE], f32)
        lanes16 = rs_full[:].rearrange("c (l e) -> (c l) e", l=2)   # [16, 6272]
        for g in range(8):
            eng = nc.sync if g % 2 == 0 else nc.gpsimd
            eng.dma_start(data_sb[16 * g:16 * g + 16, :], lanes16[:, :])

        # --- gather + mask + collapse into psum_v[128, J] -------------------
        # indirect_copy caps at 1024 valid indices -> 1024 + 16 split per round
        psum_v = pbig.tile([P, J], f32, tag="pv")
        for r in range(16):
            ic_a = gp.tile([P, 1024], f32, tag="ica")
            nc.gpsimd.indirect_copy(
                out=ic_a[:],
                data=data_sb[:],
                idxs=icidx_sb[:, 64 * r:64 * r + 64],
                i_know_ap_gather_is_preferred=True,
            )
            mk_t = mp.tile([P, 1024], bf16, tag="mk")
            nc.vector.tensor_tensor(
                out=mk_t[:],
                in0=ic_a[:],
                in1=lmask_sb[:, 1024 * r:1024 * (r + 1)],
                op=mybir.AluOpType.mult,
            )
            stat = swide_sb[:, 120 - 8 * r:248 - 8 * r]
            for j0, j1 in ((0, 512), (512, 1024)):
                nc.tensor.matmul(
                    psum_v[:, j0:j1],
                    stat,
                    mk_t[:, j0:j1],
                    start=(r == 0),
                    stop=(r == 15),
                )
        # tail: slots j in [1024, 1040) for all 16 rounds in one gather
        ic_c = gp.tile([P, 256], f32, tag="icc")
        nc.gpsimd.indirect_copy(
            out=ic_c[:],
            data=data_sb[:],
            idxs=icidx_sb[:, 1024:1040],
            i_know_ap_gather_is_preferred=True,
        )
        mk_c = mp.tile([P, 256], bf16, tag="mkc")
        nc.vector.tensor_tensor(
            out=mk_c[:],
            in0=ic_c[:],
            in1=lmask_sb[:, 16384:16640],
            op=mybir.AluOpType.mult,
        )
        for r in range(16):
            nc.tensor.matmul(
                psum_v[:, 1024:J],
                swide_sb[:, 120 - 8 * r:248 - 8 * r],
                mk_c[:, 16 * r:16 * (r + 1)],
                start=(r == 0),
                stop=(r == 15),
            )

        # --- cum = scan(V) + cross-partition offsets ------------------------
        cum_part = const.tile([P, J], f32)
        nc.vector.tensor_tensor_scan(
            out=cum_part[:],
            data0=psum_v[:],
            data1=zeros_j[:],
            initial=0.0,
            op0=mybir.AluOpType.add,
            op1=mybir.AluOpType.add,
        )
        pt_ps = psm.tile([P, 1], f32, tag="pt")
        nc.tensor.matmul(
            pt_ps[:], triex_sb[:], cum_part[:, J - 1:J], start=True, stop=True
        )
        cum_sb = const.tile([P, J], f32)
        nc.vector.tensor_scalar(
            out=cum_sb[:], in0=cum_part[:], scalar1=pt_ps[:], scalar2=None,
            op0=mybir.AluOpType.add,
        )
        nc.sync.dma_start(
            cum_dram[:].rearrange("(p j) one -> p (j one)", p=P), cum_sb[:]
        )

        # --- segment ends ----------------------------------------------------
        g1s = const.tile([P, K16 + 1], f32)
        nc.vector.memset(g1s[:, 0:1], 0.0)
        for k in range(K16):
            gt = ge.tile([P, 1], f32, tag="gt")
            nc.gpsimd.indirect_dma_start(
                out=gt[:],
                out_offset=None,
                in_=cum_dram[:],
                in_offset=IndirectOffsetOnAxis(ap=endoff_sb[:, k:k + 1], axis=0),
            )
            nc.scalar.copy(g1s[:, 1 + k:2 + k], gt[:])
        # G2 = shift(G1): sub-diagonal + corner
        g2_ps = psm.tile([P, K16], f32, tag="g2")
        nc.tensor.matmul(
            g2_ps[:], subm_sb[:], g1s[:, 1:K16 + 1], start=True, stop=False
        )
        nc.tensor.matmul(
            g2_ps[:], cornm_sb[:], g1s[:, 0:K16], start=False, stop=True
        )

        segsum = small.tile([P, K16], f32)
        nc.vector.tensor_tensor(
            out=segsum[:], in0=g1s[:, 1:K16 + 1], in1=g2_ps[:],
            op=mybir.AluOpType.subtract,
        )
        lm = small.tile([P, K16], f32)
        nc.vector.tensor_scalar_max(lm[:], lens_sb[:], 1.0)
        rec = small.tile([P, K16], f32)
        nc.vector.reciprocal(rec[:], lm[:])
        sv = small.tile([P, K16], f32)
        nc.vector.tensor_tensor(
            out=sv[:], in0=segsum[:], in1=rec[:], op=mybir.AluOpType.mult
        )

        # --- out[128k+p, l] = sv[p, k] * w_b[p, l] + b_b[p, l] ---------------
        out_sb = const.tile([P, K16, NL], f32)
        for k in range(K16):
            nc.vector.tensor_scalar(
                out=out_sb[:, k, :], in0=w_b[:],
                scalar1=sv[:, k:k + 1], scalar2=None,
                op0=mybir.AluOpType.mult,
            )
            nc.vector.tensor_tensor(
                out=out_sb[:, k, :], in0=out_sb[:, k, :], in1=b_b[:],
                op=mybir.AluOpType.add,
            )
        nc.sync.dma_start(out[:].rearrange("(k p) l -> p k l", p=P), out_sb[:])

    _split_waits(nc)
    return nc


_PROGRAM = None


def _get_program():
    global _PROGRAM
    if _PROGRAM is None:
        _PROGRAM = build_program()
    return _PROGRAM


def _host_prep(tokens, segment_ids, word_vectors, hidden_w, hidden_b):
    """Integer-only preprocessing: shard + slot/lane/end index tensors."""
    tokens = np.asarray(tokens)
    segment_ids = np.asarray(segment_ids)
    wv = np.asarray(word_vectors, dtype=np.float32)
    hw = np.asarray(hidden_w, dtype=np.float32)
    hbv = np.asarray(hidden_b, dtype=np.float32)

    wv_pad = np.zeros((VOC_PAD, D), dtype=np.float32)
    wv_pad[:VOC] = wv
    hwT = np.ascontiguousarray(hw.T)
    hbr = hbv.reshape(1, NL)

    triex = np.triu(np.ones((P, P), dtype=np.float32), 1)       # pi < po
    subm = np.zeros((P, P), dtype=np.float32)
    subm[np.arange(P - 1), np.arange(1, P)] = 1.0               # po = pi+1
    cornm = np.zeros((P, P), dtype=np.float32)
    cornm[P - 1, 0] = 1.0                                       # po=0 <- pi=127
    swide = np.zeros((P, 248), dtype=np.float32)
    swide[np.arange(P), 120 + np.arange(P) // 16] = 1.0
    swide = swide.astype(ml_dtypes.bfloat16)

    cuts = np.searchsorted(segment_ids, np.arange(NCORES + 1) * SENT_PER_CORE)
    in_maps = []
    for c in range(NCORES):
        lo, hi = int(cuts[c]), int(cuts[c + 1])
        n = hi - lo
        assert n <= SLOTS, f"shard {c}: {n} > {SLOTS}"
        toks = tokens[lo:hi].astype(np.int64)
        segs = (segment_ids[lo:hi] - c * SENT_PER_CORE).astype(np.int64)

        q = np.arange(n)
        p = q // J                 # partition-major token layout
        j = q % J
        g = p % 8
        r = p // 8
        i = r * J + j
        lane = toks // LANE
        e = toks % LANE

        icidx_a = np.zeros((P, 1040), dtype=np.uint16)
        lo_j = j < 1024
        icidx_a[16 * g[lo_j] + j[lo_j] % 16,
                64 * r[lo_j] + j[lo_j] // 16] = e[lo_j].astype(np.uint16)
        hi_j = ~lo_j
        it = r[hi_j] * 16 + (j[hi_j] - 1024)       # tail slot index [0, 256)
        icidx_a[16 * g[hi_j] + it % 16, 1024 + it // 16] = e[hi_j].astype(np.uint16)
        lmask_a = np.zeros((P, IC), dtype=np.float32)
        im = np.where(j < 1024, 1024 * r + j, 16384 + r * 16 + (j - 1024))
        lmask_a[16 * g + lane, im] = 1.0

        ends_q = np.searchsorted(segs, np.arange(SENT_PER_CORE), side="right") - 1
        lens_a = np.bincount(segs, minlength=SENT_PER_CORE).astype(np.float32)
        assert lens_a.min() >= 1, f"shard {c} has empty segments"
        endoff_a = ends_q.reshape(K16, P).T.astype(np.int32).copy()
        lens_t = lens_a.reshape(K16, P).T.astype(np.float32).copy()

        in_maps.append(
            {
                "wvs": wv_pad[c * VSHARD:(c + 1) * VSHARD],
                "icidx": icidx_a,
                "lmask": lmask_a,
                "endoff": endoff_a,
                "lens": lens_t,
                "swide": swide,
                "triex": triex,
                "subm": subm,
                "cornm": cornm,
                "hwT": hwT,
                "hb": hbr,
            }
        )
    return in_maps


def kernel(tokens, segment_ids, word_vectors, hidden_w, hidden_b):
    from concourse.bass_utils import run_bass_kernel_spmd

    in_maps = _host_prep(tokens, segment_ids, word_vectors, hidden_w, hidden_b)
    nc = _get_program()
    res = run_bass_kernel_spmd(nc, in_maps, list(range(NCORES)))
    return np.concatenate([res.results[c]["out"] for c in range(NCORES)], axis=0)


# revision 5
# speedup vs baseline: 1.1763x; 1.0261x over previous
"""Trainium2 Bass kernel for nn_Net_28544352649361 (segment_reduce), v2.

Per core (2048 sentences, <=133120 tokens laid out partition-major:
token q -> (p = q // 1040, j = q % 1040)):
  1. rowsum[v] = sum_d wv[v, d]: reduce own 12544-row vocab shard (6.4MB
     dense read + DVE reduce), AllGather -> full [100352] table in DRAM.
  2. Lane tables: SBUF data[p, e] = rowsum[6272*(p%16) + e] via one
     broadcast-AP DMA; each 16-partition GPSIMD group holds the whole vocab.
  3. Gather: 8x indirect_copy fetches entry e(v)=v%6272 on all 16 lanes of
     the owning group; host bf16... f32 mask keeps lane v//6272; bf16
     mask-product collapses via 0/1-stationary matmuls that route slot
     (g, r*1040+j) -> psum_v[8r+g, j] = V in token order.
  4. cum: DVE tensor_tensor_scan along free dim + cross-partition exclusive
     prefix (tri matmul) + per-partition offset add; cum -> DRAM (flat = q).
  5. Segment sums: 16 indirect gathers of cum at segment-end positions;
     predecessor ends via shift matmuls; sv = (G1 - G2) / max(len, 1).
  6. out[s, l] = sv[s] * rowsum(hidden_w)[l] + hb[l].
Host prep is integer-only: shard cuts, slot/lane indices, end offsets, lens.
"""

import sys

sys.path.insert(0, "/opt/trn_rl_repo")

from contextlib import ExitStack

import dataclasses
import numpy as np
import ml_dtypes

import concourse.bass as bass
import concourse.tile as tile
from concourse import mybir
from concourse.bass import IndirectOffsetOnAxis
from concourse.vector_clock import ScopedClock

P = 128
J = 1040                  # free-dim tokens per partition; slots = 128*J
SLOTS = P * J             # 133120 >= max shard (131371)
IC = 16 * J               # indirect_copy output columns (8 groups x 2 rounds)
NCHUNK = 8
CCOL = IC // NCHUNK       # 2080 ic-columns per chunk (= 2 rounds of J)
D = 128
NL = 128
NSENT = 16384
NCORES = 8
SENT_PER_CORE = NSENT // NCORES   # 2048
K16 = SENT_PER_CORE // P          # 16
VOC = 100000
VOC_PAD = 100352
VSHARD = VOC_PAD // NCORES        # 12544
LANE = VOC_PAD // 16              # 6272
U = VSHARD // P                   # 98 rows per partition in rowsum build

_num_splits = [0]


# ---------------------------------------------------------------------------
# Workarounds for this walrus build (accepts only ONE sync-wait per
# instruction) and Tile's drain path.
# ---------------------------------------------------------------------------
def _split_drain_and_barrier(self, tick_clock, wait_clock):
    nc = self.nc
    drain_inst = nc.sync.drain()
    wait_clock.add_sem_waits(
        drain_inst.ins, ScopedClock({None: tick_clock.global_clock})
    )
    mi = drain_inst.ins
    si = mi.sync_info
    if si is not None and si.on_wait is not None and len(si.on_wait) > 1:
        waits = list(si.on_wait)
        si.on_wait = waits[:1]
        for w in waits[1:]:
            extra = nc.sync.drain()
            emi = extra.ins
            esi = emi.sync_info
            if esi is None:
                emi.sync_info = mybir.SyncInfo(on_wait=[w], on_update=[])
            else:
                esi.on_wait = [w]
    nc.all_engine_barrier()
    assert self.sems is not None
    popped = nc._tile_sem_poison_stack.pop()
    assert popped is self._sem_poison
    nc.clear_and_free_semaphores(list(self.sems.allocated().values()))
    nc.all_engine_barrier()


def _apply_patches():
    if getattr(tile, "_segred_patched", False):
        return
    tile.TileContext._drain_and_barrier = _split_drain_and_barrier
    tile._segred_patched = True


def _split_waits(nc):
    """Hoist surplus sync-waits onto same-engine NoOps placed just before the
    waiter; the engine sequencer executes them in order."""
    import bass_rust

    for f in nc.m.functions:
        for bb in f.blocks:
            new_list = []
            changed = False
            for inst in bb.instructions:
                si = inst.sync_info
                if si is not None and si.on_wait is not None and len(si.on_wait) > 1:
                    waits = list(si.on_wait)
                    si.on_wait = waits[-1:]
                    for w in waits[:-1]:
                        _num_splits[0] += 1
                        nop = bass_rust.InstNoOp(
                            name=f"WSPLIT-{_num_splits[0]}", ins=[], outs=[]
                        )
                        nop.engine = inst.engine
                        nop.sync_info = mybir.SyncInfo(on_wait=[w], on_update=[])
                        new_list.append(nop)
                    changed = True
                new_list.append(inst)
            if changed:
                bb.instructions = new_list


# ---------------------------------------------------------------------------
# Device program (identical for all cores; per-core data via in_maps)
# ---------------------------------------------------------------------------
def build_program():
    _apply_patches()
    nc = bass.Bass(num_devices=NCORES)
    f32 = mybir.dt.float32
    i32 = mybir.dt.int32
    u16 = mybir.dt.uint16
    bf16 = mybir.dt.bfloat16

    wvs = nc.declare_dram_parameter("wvs", [VSHARD, D], f32, isOutput=False)
    icidx = nc.declare_dram_parameter("icidx", [P, 1040], u16, isOutput=False)
    lmask = nc.declare_dram_parameter("lmask", [P, IC], bf16, isOutput=False)
    endoff = nc.declare_dram_parameter("endoff", [P, K16], i32, isOutput=False)
    lens = nc.declare_dram_parameter("lens", [P, K16], f32, isOutput=False)
    swide = nc.declare_dram_parameter("swide", [P, 248], bf16, isOutput=False)
    triex = nc.declare_dram_parameter("triex", [P, P], f32, isOutput=False)
    subm = nc.declare_dram_parameter("subm", [P, P], f32, isOutput=False)
    cornm = nc.declare_dram_parameter("cornm", [P, P], f32, isOutput=False)
    hwT = nc.declare_dram_parameter("hwT", [D, NL], f32, isOutput=False)
    hb = nc.declare_dram_parameter("hb", [1, NL], f32, isOutput=False)
    out = nc.declare_dram_parameter("out", [SENT_PER_CORE, NL], f32, isOutput=True)

    rs_shard = nc.dram_tensor("rs_shard", [1, VSHARD], bf16)
    rs_full = nc.dram_tensor("rs_full", [NCORES, VSHARD], bf16)
    cum_dram = nc.dram_tensor("cum_dram", [SLOTS, 1], f32)

    with ExitStack() as ctx:
        tc = ctx.enter_context(tile.TileContext(nc))
        const = ctx.enter_context(tc.tile_pool(name="const", bufs=1))
        gp = ctx.enter_context(tc.tile_pool(name="g", bufs=2))
        mp = ctx.enter_context(tc.tile_pool(name="m", bufs=2))
        small = ctx.enter_context(tc.tile_pool(name="small", bufs=3))
        ge = ctx.enter_context(tc.tile_pool(name="ge", bufs=16))
        pbig = ctx.enter_context(tc.tile_pool(name="pb", bufs=1, space="PSUM"))
        psm = ctx.enter_context(tc.tile_pool(name="psm", bufs=1, space="PSUM"))

        # --- rowsum shard first: its loads/reduce/AllGather gate everything -
        rs_sb = const.tile([P, U], f32)
        wv_view = wvs[:].rearrange("(p u) d -> p u d", p=P)
        bounds = [(0, 24), (24, 48), (48, 72), (72, U)]
        for u0, u1 in bounds:
            wvt = gp.tile([P, u1 - u0, D], f32, tag="wv")
            nc.sync.dma_start(wvt[:], wv_view[0:P, u0:u1, :])
            nc.vector.tensor_reduce(
                rs_sb[:, u0:u1],
                wvt[:],
                axis=mybir.AxisListType.X,
                op=mybir.AluOpType.add,
            )
        rs_bf = const.tile([P, U], bf16)
        nc.scalar.copy(rs_bf[:], rs_sb[:])
        nc.sync.dma_start(
            rs_shard[:].rearrange("one (p u) -> p (one u)", p=P), rs_bf[:]
        )
        nc.gpsimd.collective_compute(
            "AllGather",
            mybir.AluOpType.bypass,
            replica_groups=[list(range(NCORES))],
            ins=[rs_shard[:].opt()],
            outs=[rs_full[:].opt()],
        )

        # --- constants / small inputs (overlap the AllGather window) --------
        icidx_sb = const.tile([P, 1040], u16)
        nc.sync.dma_start(icidx_sb[:], icidx[:])
        lmask_sb = const.tile([P, IC], bf16)
        for q0 in range(0, IC, IC // 4):
            nc.sync.dma_start(
                lmask_sb[:, q0:q0 + IC // 4], lmask[:, q0:q0 + IC // 4]
            )
        endoff_sb = const.tile([P, K16], i32)
        nc.sync.dma_start(endoff_sb[:], endoff[:])
        lens_sb = const.tile([P, K16], f32)
        nc.sync.dma_start(lens_sb[:], lens[:])
        swide_sb = const.tile([P, 248], bf16)
        nc.sync.dma_start(swide_sb[:], swide[:])
        triex_sb = const.tile([P, P], f32)
        nc.sync.dma_start(triex_sb[:], triex[:])
        subm_sb = const.tile([P, P], f32)
        nc.sync.dma_start(subm_sb[:], subm[:])
        cornm_sb = const.tile([P, P], f32)
        nc.sync.dma_start(cornm_sb[:], cornm[:])
        hwT_sb = const.tile([D, NL], f32)
        nc.sync.dma_start(hwT_sb[:], hwT[:])
        hb_sb = const.tile([1, NL], f32)
        nc.sync.dma_start(hb_sb[:], hb[:])
        ones_p = const.tile([P, 1], f32)
        nc.vector.memset(ones_p[:], 1.0)
        ones_1 = const.tile([1, P], f32)
        nc.vector.memset(ones_1[:], 1.0)
        zeros_j = const.tile([P, J], f32)
        nc.vector.memset(zeros_j[:], 0.0)

        # --- w_b / b_b broadcast --------------------------------------------
        wrow_ps = psm.tile([1, NL], f32, tag="ps1")
        nc.tensor.matmul(wrow_ps[:], ones_p[:], hwT_sb[:], start=True, stop=True)
        wrow = small.tile([1, NL], f32)
        nc.scalar.copy(wrow[:], wrow_ps[:])
        wb_ps = psm.tile([P, NL], f32, tag="ps2")
        nc.tensor.matmul(wb_ps[:], ones_1[:], wrow[:], start=True, stop=True)
        w_b = const.tile([P, NL], f32)
        nc.scalar.copy(w_b[:], wb_ps[:])
        bb_ps = psm.tile([P, NL], f32, tag="ps2")
        nc.tensor.matmul(bb_ps[:], ones_1[:], hb_sb[:], start=True, stop=True)
        b_b = const.tile([P, NL], f32)
        nc.scalar.copy(b_b[:], bb_ps[:])
        lm = small.tile([P, K16], f32)
        nc.vector.tensor_scalar_max(lm[:], lens_sb[:], 1.0)
        rec = const.tile([P, K16], f32)
        nc.vector.reciprocal(rec[:], lm[:])

        # --- lane tables: data[p, e] = rowsum[6272*(p%16) + e] --------------
        data_sb = const.tile([P, LANE], bf16)
        lanes16 = rs_full[:].rearrange("c (l e) -> (c l) e", l=2)   # [16, 6272]
        for g in range(8):
            eng = nc.sync if g % 2 == 0 else nc.gpsimd
            eng.dma_start(data_sb[16 * g:16 * g + 16, :], lanes16[:, :])

        # --- gather + mask + collapse into psum_v[128, J] -------------------
        # indirect_copy caps at 1024 valid indices -> 1024 + 16 split per round
        psum_v = pbig.tile([P, J], f32, tag="pv")
        for r in range(16):
            ic_a = gp.tile([P, 1024], bf16, tag="ica")
            nc.gpsimd.indirect_copy(
                out=ic_a[:],
                data=data_sb[:],
                idxs=icidx_sb[:, 64 * r:64 * r + 64],
                i_know_ap_gather_is_preferred=True,
            )
            mk_t = mp.tile([P, 1024], bf16, tag="mk")
            nc.vector.tensor_tensor(
                out=mk_t[:],
                in0=ic_a[:],
                in1=lmask_sb[:, 1024 * r:1024 * (r + 1)],
                op=mybir.AluOpType.mult,
            )
            stat = swide_sb[:, 120 - 8 * r:248 - 8 * r]
            for j0, j1 in ((0, 512), (512, 1024)):
                nc.tensor.matmul(
                    psum_v[:, j0:j1],
                    stat,
                    mk_t[:, j0:j1],
                    start=(r == 0),
                    stop=(r == 15),
                )
        # tail: slots j in [1024, 1040) for all 16 rounds in one gather
        ic_c = gp.tile([P, 256], bf16, tag="icc")
        nc.gpsimd.indirect_copy(
            out=ic_c[:],
            data=data_sb[:],
            idxs=icidx_sb[:, 1024:1040],
            i_know_ap_gather_is_preferred=True,
        )
        mk_c = mp.tile([P, 256], bf16, tag="mkc")
        nc.vector.tensor_tensor(
            out=mk_c[:],
            in0=ic_c[:],
            in1=lmask_sb[:, 16384:16640],
            op=mybir.AluOpType.mult,
        )
        for r in range(16):
            nc.tensor.matmul(
                psum_v[:, 1024:J],
                swide_sb[:, 120 - 8 * r:248 - 8 * r],
                mk_c[:, 16 * r:16 * (r + 1)],
                start=(r == 0),
                stop=(r == 15),
            )

        # --- cum = scan(V) + cross-partition offsets ------------------------
        cum_part = const.tile([P, J], f32)
        nc.vector.tensor_tensor_scan(
            out=cum_part[:],
            data0=psum_v[:],
            data1=zeros_j[:],
            initial=0.0,
            op0=mybir.AluOpType.add,
            op1=mybir.AluOpType.add,
        )
        pt_ps = psm.tile([P, 1], f32, tag="pt")
        nc.tensor.matmul(
            pt_ps[:], triex_sb[:], cum_part[:, J - 1:J], start=True, stop=True
        )
        cum_sb = const.tile([P, J], f32)
        nc.vector.tensor_scalar(
            out=cum_sb[:], in0=cum_part[:], scalar1=pt_ps[:], scalar2=None,
            op0=mybir.AluOpType.add,
        )
        nc.sync.dma_start(
            cum_dram[:].rearrange("(p j) one -> p (j one)", p=P), cum_sb[:]
        )

        # --- segment ends: per-k pipelined gather -> shift -> sv -> out ------
        g1s = const.tile([P, K16 + 1], f32)
        nc.vector.memset(g1s[:, 0:1], 0.0)
        out_sb = const.tile([P, K16, NL], f32)
        out_view = out[:].rearrange("(k p) l -> p k l", p=P)
        sv = const.tile([P, K16], f32)
        for k in range(K16):
            gt = ge.tile([P, 1], f32, tag="gt")
            nc.gpsimd.indirect_dma_start(
                out=gt[:],
                out_offset=None,
                in_=cum_dram[:],
                in_offset=IndirectOffsetOnAxis(ap=endoff_sb[:, k:k + 1], axis=0),
            )
            nc.scalar.copy(g1s[:, 1 + k:2 + k], gt[:])
            g2k = psm.tile([P, 1], f32, tag="g2k")
            nc.tensor.matmul(
                g2k[:], subm_sb[:], g1s[:, 1 + k:2 + k], start=True, stop=False
            )
            nc.tensor.matmul(
                g2k[:], cornm_sb[:], g1s[:, k:1 + k], start=False, stop=True
            )
            nc.vector.tensor_tensor(
                out=sv[:, k:k + 1], in0=g1s[:, 1 + k:2 + k], in1=g2k[:],
                op=mybir.AluOpType.subtract,
            )
            nc.vector.tensor_tensor(
                out=sv[:, k:k + 1], in0=sv[:, k:k + 1], in1=rec[:, k:k + 1],
                op=mybir.AluOpType.mult,
            )
            nc.vector.tensor_scalar(
                out=out_sb[:, k, :], in0=w_b[:],
                scalar1=sv[:, k:k + 1], scalar2=None,
                op0=mybir.AluOpType.mult,
            )
            nc.vector.tensor_tensor(
                out=out_sb[:, k, :], in0=out_sb[:, k, :], in1=b_b[:],
                op=mybir.AluOpType.add,
            )
            if k == K16 // 2 - 1:
                nc.sync.dma_start(
                    out_view[0:P, 0:K16 // 2, :], out_sb[:, 0:K16 // 2, :]
                )
        nc.sync.dma_start(
            out_view[0:P, K16 // 2:K16, :], out_sb[:, K16 // 2:K16, :]
        )

    _split_waits(nc)
    return nc


_PROGRAM = None


def _get_program():
    global _PROGRAM
    if _PROGRAM is None:
        _PROGRAM = build_program()
    return _PROGRAM


def _host_prep(tokens, segment_ids, word_vectors, hidden_w, hidden_b):
    """Integer-only preprocessing: shard + slot/lane/end index tensors."""
    tokens = np.asarray(tokens)
    segment_ids = np.asarray(segment_ids)
    wv = np.asarray(word_vectors, dtype=np.float32)
    hw = np.asarray(hidden_w, dtype=np.float32)
    hbv = np.asarray(hidden_b, dtype=np.float32)

    wv_pad = np.zeros((VOC_PAD, D), dtype=np.float32)
    wv_pad[:VOC] = wv
    hwT = np.ascontiguousarray(hw.T)
    hbr = hbv.reshape(1, NL)

    triex = np.triu(np.ones((P, P), dtype=np.float32), 1)       # pi < po
    subm = np.zeros((P, P), dtype=np.float32)
    subm[np.arange(P - 1), np.arange(1, P)] = 1.0               # po = pi+1
    cornm = np.zeros((P, P), dtype=np.float32)
    cornm[P - 1, 0] = 1.0                                       # po=0 <- pi=127
    swide = np.zeros((P, 248), dtype=np.float32)
    swide[np.arange(P), 120 + np.arange(P) // 16] = 1.0
    swide = swide.astype(ml_dtypes.bfloat16)

    cuts = np.searchsorted(segment_ids, np.arange(NCORES + 1) * SENT_PER_CORE)
    in_maps = []
    for c in range(NCORES):
        lo, hi = int(cuts[c]), int(cuts[c + 1])
        n = hi - lo
        assert n <= SLOTS, f"shard {c}: {n} > {SLOTS}"
        toks = tokens[lo:hi].astype(np.int64)
        segs = (segment_ids[lo:hi] - c * SENT_PER_CORE).astype(np.int64)

        q = np.arange(n)
        p = q // J                 # partition-major token layout
        j = q % J
        g = p % 8
        r = p // 8
        i = r * J + j
        lane = toks // LANE
        e = toks % LANE

        icidx_a = np.zeros((P, 1040), dtype=np.uint16)
        lo_j = j < 1024
        icidx_a[16 * g[lo_j] + j[lo_j] % 16,
                64 * r[lo_j] + j[lo_j] // 16] = e[lo_j].astype(np.uint16)
        hi_j = ~lo_j
        it = r[hi_j] * 16 + (j[hi_j] - 1024)       # tail slot index [0, 256)
        icidx_a[16 * g[hi_j] + it % 16, 1024 + it // 16] = e[hi_j].astype(np.uint16)
        lmask_a = np.zeros((P, IC), dtype=np.float32)
        im = np.where(j < 1024, 1024 * r + j, 16384 + r * 16 + (j - 1024))
        lmask_a[16 * g + lane, im] = 1.0
        lmask_a = lmask_a.astype(ml_dtypes.bfloat16)

        ends_q = np.searchsorted(segs, np.arange(SENT_PER_CORE), side="right") - 1
        lens_a = np.bincount(segs, minlength=SENT_PER_CORE).astype(np.float32)
        assert lens_a.min() >= 1, f"shard {c} has empty segments"
        endoff_a = ends_q.reshape(K16, P).T.astype(np.int32).copy()
        lens_t = lens_a.reshape(K16, P).T.astype(np.float32).copy()

        in_maps.append(
            {
                "wvs": wv_pad[c * VSHARD:(c + 1) * VSHARD],
                "icidx": icidx_a,
                "lmask": lmask_a,
                "endoff": endoff_a,
                "lens": lens_t,
                "swide": swide,
                "triex": triex,
                "subm": subm,
                "cornm": cornm,
                "hwT": hwT,
                "hb": hbr,
            }
        )
    return in_maps


def kernel(tokens, segment_ids, word_vectors, hidden_w, hidden_b):
    from concourse.bass_utils import run_bass_kernel_spmd

    in_maps = _host_prep(tokens, segment_ids, word_vectors, hidden_w, hidden_b)
    nc = _get_program()
    res = run_bass_kernel_spmd(nc, in_maps, list(range(NCORES)))
    return np.concatenate([res.results[c]["out"] for c in range(NCORES)], axis=0)


# revision 6
# speedup vs baseline: 1.2198x; 1.0369x over previous
"""Trainium2 Bass kernel for nn_Net_28544352649361 (segment_reduce), v2.

Per core (2048 sentences, <=133120 tokens laid out partition-major:
token q -> (p = q // 1040, j = q % 1040)):
  1. rowsum[v] = sum_d wv[v, d]: reduce own 12544-row vocab shard (6.4MB
     dense read + DVE reduce), AllGather -> full [100352] table in DRAM.
  2. Lane tables: SBUF data[p, e] = rowsum[6272*(p%16) + e] via one
     broadcast-AP DMA; each 16-partition GPSIMD group holds the whole vocab.
  3. Gather: 8x indirect_copy fetches entry e(v)=v%6272 on all 16 lanes of
     the owning group; host bf16... f32 mask keeps lane v//6272; bf16
     mask-product collapses via 0/1-stationary matmuls that route slot
     (g, r*1040+j) -> psum_v[8r+g, j] = V in token order.
  4. cum: DVE tensor_tensor_scan along free dim + cross-partition exclusive
     prefix (tri matmul) + per-partition offset add; cum -> DRAM (flat = q).
  5. Segment sums: 16 indirect gathers of cum at segment-end positions;
     predecessor ends via shift matmuls; sv = (G1 - G2) / max(len, 1).
  6. out[s, l] = sv[s] * rowsum(hidden_w)[l] + hb[l].
Host prep is integer-only: shard cuts, slot/lane indices, end offsets, lens.
"""

import sys

sys.path.insert(0, "/opt/trn_rl_repo")

from contextlib import ExitStack

import dataclasses
import numpy as np
import ml_dtypes

import concourse.bass as bass
import concourse.tile as tile
from concourse import mybir
from concourse.bass import IndirectOffsetOnAxis
from concourse.vector_clock import ScopedClock

P = 128
J = 1040                  # free-dim tokens per partition; slots = 128*J
SLOTS = P * J             # 133120 >= max shard (131371)
IC = 16 * J               # indirect_copy output columns (8 groups x 2 rounds)
NCHUNK = 8
CCOL = IC // NCHUNK       # 2080 ic-columns per chunk (= 2 rounds of J)
D = 128
NL = 128
NSENT = 16384
NCORES = 8
SENT_PER_CORE = NSENT // NCORES   # 2048
K16 = SENT_PER_CORE // P          # 16
VOC = 100000
VOC_PAD = 100352
VSHARD = VOC_PAD // NCORES        # 12544
LANE = VOC_PAD // 16              # 6272
U = VSHARD // P                   # 98 rows per partition in rowsum build

_num_splits = [0]


# ---------------------------------------------------------------------------
# Workarounds for this walrus build (accepts only ONE sync-wait per
# instruction) and Tile's drain path.
# ---------------------------------------------------------------------------
def _split_drain_and_barrier(self, tick_clock, wait_clock):
    nc = self.nc
    drain_inst = nc.sync.drain()
    wait_clock.add_sem_waits(
        drain_inst.ins, ScopedClock({None: tick_clock.global_clock})
    )
    mi = drain_inst.ins
    si = mi.sync_info
    if si is not None and si.on_wait is not None and len(si.on_wait) > 1:
        waits = list(si.on_wait)
        si.on_wait = waits[:1]
        for w in waits[1:]:
            extra = nc.sync.drain()
            emi = extra.ins
            esi = emi.sync_info
            if esi is None:
                emi.sync_info = mybir.SyncInfo(on_wait=[w], on_update=[])
            else:
                esi.on_wait = [w]
    nc.all_engine_barrier()
    assert self.sems is not None
    popped = nc._tile_sem_poison_stack.pop()
    assert popped is self._sem_poison
    nc.clear_and_free_semaphores(list(self.sems.allocated().values()))
    nc.all_engine_barrier()


def _apply_patches():
    if getattr(tile, "_segred_patched", False):
        return
    tile.TileContext._drain_and_barrier = _split_drain_and_barrier
    tile._segred_patched = True


def _split_waits(nc):
    """Hoist surplus sync-waits onto same-engine NoOps placed just before the
    waiter; the engine sequencer executes them in order."""
    import bass_rust

    for f in nc.m.functions:
        for bb in f.blocks:
            new_list = []
            changed = False
            for inst in bb.instructions:
                si = inst.sync_info
                if si is not None and si.on_wait is not None and len(si.on_wait) > 1:
                    waits = list(si.on_wait)
                    si.on_wait = waits[-1:]
                    for w in waits[:-1]:
                        _num_splits[0] += 1
                        nop = bass_rust.InstNoOp(
                            name=f"WSPLIT-{_num_splits[0]}", ins=[], outs=[]
                        )
                        nop.engine = inst.engine
                        nop.sync_info = mybir.SyncInfo(on_wait=[w], on_update=[])
                        new_list.append(nop)
                    changed = True
                new_list.append(inst)
            if changed:
                bb.instructions = new_list


# ---------------------------------------------------------------------------
# Device program (identical for all cores; per-core data via in_maps)
# ---------------------------------------------------------------------------
def build_program():
    _apply_patches()
    nc = bass.Bass(num_devices=NCORES)
    f32 = mybir.dt.float32
    i32 = mybir.dt.int32
    u16 = mybir.dt.uint16
    bf16 = mybir.dt.bfloat16

    wvs = nc.declare_dram_parameter("wvs", [VSHARD, D], f32, isOutput=False)
    icidx = nc.declare_dram_parameter("icidx", [P, 1040], u16, isOutput=False)
    lmask = nc.declare_dram_parameter("lmask", [P, IC], bf16, isOutput=False)
    endoff = nc.declare_dram_parameter("endoff", [P, K16], i32, isOutput=False)
    lens = nc.declare_dram_parameter("lens", [P, K16], f32, isOutput=False)
    swide = nc.declare_dram_parameter("swide", [P, 248], bf16, isOutput=False)
    triex = nc.declare_dram_parameter("triex", [P, P], f32, isOutput=False)
    subm = nc.declare_dram_parameter("subm", [P, P], f32, isOutput=False)
    cornm = nc.declare_dram_parameter("cornm", [P, P], f32, isOutput=False)
    hwT = nc.declare_dram_parameter("hwT", [D, NL], f32, isOutput=False)
    hb = nc.declare_dram_parameter("hb", [1, NL], f32, isOutput=False)
    out = nc.declare_dram_parameter("out", [SENT_PER_CORE, NL], f32, isOutput=True)

    rs_shard = nc.dram_tensor("rs_shard", [1, VSHARD], bf16)
    rs_full = nc.dram_tensor("rs_full", [NCORES, VSHARD], bf16)
    cum_dram = nc.dram_tensor("cum_dram", [SLOTS, 1], f32)

    with ExitStack() as ctx:
        tc = ctx.enter_context(tile.TileContext(nc))
        const = ctx.enter_context(tc.tile_pool(name="const", bufs=1))
        gp = ctx.enter_context(tc.tile_pool(name="g", bufs=3))
        mp = ctx.enter_context(tc.tile_pool(name="m", bufs=2))
        small = ctx.enter_context(tc.tile_pool(name="small", bufs=3))
        ge = ctx.enter_context(tc.tile_pool(name="ge", bufs=16))
        pbig = ctx.enter_context(tc.tile_pool(name="pb", bufs=1, space="PSUM"))
        psm = ctx.enter_context(tc.tile_pool(name="psm", bufs=1, space="PSUM"))

        # --- rowsum shard first: its loads/reduce/AllGather gate everything -
        rs_sb = const.tile([P, U], f32)
        wv_view = wvs[:].rearrange("(p u) d -> p u d", p=P)
        bounds = [(0, 24), (24, 48), (48, 72), (72, U)]
        for u0, u1 in bounds:
            wvt = gp.tile([P, u1 - u0, D], f32, tag="wv")
            nc.sync.dma_start(wvt[:], wv_view[0:P, u0:u1, :])
            nc.vector.tensor_reduce(
                rs_sb[:, u0:u1],
                wvt[:],
                axis=mybir.AxisListType.X,
                op=mybir.AluOpType.add,
            )
        rs_bf = const.tile([P, U], bf16)
        nc.scalar.copy(rs_bf[:], rs_sb[:])
        nc.sync.dma_start(
            rs_shard[:].rearrange("one (p u) -> p (one u)", p=P), rs_bf[:]
        )
        nc.gpsimd.collective_compute(
            "AllGather",
            mybir.AluOpType.bypass,
            replica_groups=[list(range(NCORES))],
            ins=[rs_shard[:].opt()],
            outs=[rs_full[:].opt()],
        )

        # --- constants / small inputs (overlap the AllGather window) --------
        icidx_sb = const.tile([P, 1040], u16)
        nc.sync.dma_start(icidx_sb[:], icidx[:])
        lmask_sb = const.tile([P, IC], bf16)
        for q0 in range(0, IC, IC // 4):
            nc.sync.dma_start(
                lmask_sb[:, q0:q0 + IC // 4], lmask[:, q0:q0 + IC // 4]
            )
        endoff_sb = const.tile([P, K16], i32)
        nc.sync.dma_start(endoff_sb[:], endoff[:])
        lens_sb = const.tile([P, K16], f32)
        nc.sync.dma_start(lens_sb[:], lens[:])
        swide_sb = const.tile([P, 248], bf16)
        nc.sync.dma_start(swide_sb[:], swide[:])
        triex_sb = const.tile([P, P], f32)
        nc.sync.dma_start(triex_sb[:], triex[:])
        subm_sb = const.tile([P, P], f32)
        nc.sync.dma_start(subm_sb[:], subm[:])
        cornm_sb = const.tile([P, P], f32)
        nc.sync.dma_start(cornm_sb[:], cornm[:])
        hwT_sb = const.tile([D, NL], f32)
        nc.sync.dma_start(hwT_sb[:], hwT[:])
        hb_sb = const.tile([1, NL], f32)
        nc.sync.dma_start(hb_sb[:], hb[:])
        ones_p = const.tile([P, 1], f32)
        nc.vector.memset(ones_p[:], 1.0)
        ones_1 = const.tile([1, P], f32)
        nc.vector.memset(ones_1[:], 1.0)
        zeros_j = const.tile([P, J], f32)
        nc.vector.memset(zeros_j[:], 0.0)

        # --- w_b / b_b broadcast --------------------------------------------
        wrow_ps = psm.tile([1, NL], f32, tag="ps1")
        nc.tensor.matmul(wrow_ps[:], ones_p[:], hwT_sb[:], start=True, stop=True)
        wrow = small.tile([1, NL], f32)
        nc.scalar.copy(wrow[:], wrow_ps[:])
        wb_ps = psm.tile([P, NL], f32, tag="ps2")
        nc.tensor.matmul(wb_ps[:], ones_1[:], wrow[:], start=True, stop=True)
        w_b = const.tile([P, NL], f32)
        nc.scalar.copy(w_b[:], wb_ps[:])
        bb_ps = psm.tile([P, NL], f32, tag="ps2")
        nc.tensor.matmul(bb_ps[:], ones_1[:], hb_sb[:], start=True, stop=True)
        b_b = const.tile([P, NL], f32)
        nc.scalar.copy(b_b[:], bb_ps[:])
        lm = small.tile([P, K16], f32)
        nc.vector.tensor_scalar_max(lm[:], lens_sb[:], 1.0)
        rec = const.tile([P, K16], f32)
        nc.vector.reciprocal(rec[:], lm[:])

        # --- lane tables: data[p, e] = rowsum[6272*(p%16) + e] --------------
        data_sb = const.tile([P, LANE], bf16)
        lanes16 = rs_full[:].rearrange("c (l e) -> (c l) e", l=2)   # [16, 6272]
        for g in range(8):
            eng = (nc.sync, nc.gpsimd, nc.scalar)[g % 3]
            eng.dma_start(data_sb[16 * g:16 * g + 16, :], lanes16[:, :])

        # --- gather + mask + collapse into psum_v[128, J] -------------------
        # indirect_copy caps at 1024 valid indices -> 1024 + 16 split per round
        psum_v = pbig.tile([P, J], f32, tag="pv")
        for r in range(16):
            ic_a = gp.tile([P, 1024], bf16, tag="ica")
            nc.gpsimd.indirect_copy(
                out=ic_a[:],
                data=data_sb[:],
                idxs=icidx_sb[:, 64 * r:64 * r + 64],
                i_know_ap_gather_is_preferred=True,
            )
            mk_t = mp.tile([P, 1024], bf16, tag="mk")
            nc.vector.tensor_tensor(
                out=mk_t[:],
                in0=ic_a[:],
                in1=lmask_sb[:, 1024 * r:1024 * (r + 1)],
                op=mybir.AluOpType.mult,
            )
            stat = swide_sb[:, 120 - 8 * r:248 - 8 * r]
            for j0, j1 in ((0, 512), (512, 1024)):
                nc.tensor.matmul(
                    psum_v[:, j0:j1],
                    stat,
                    mk_t[:, j0:j1],
                    start=(r == 0),
                    stop=(r == 15),
                )
        # tail: slots j in [1024, 1040) for all 16 rounds in one gather
        ic_c = gp.tile([P, 256], bf16, tag="icc")
        nc.gpsimd.indirect_copy(
            out=ic_c[:],
            data=data_sb[:],
            idxs=icidx_sb[:, 1024:1040],
            i_know_ap_gather_is_preferred=True,
        )
        mk_c = mp.tile([P, 256], bf16, tag="mkc")
        nc.vector.tensor_tensor(
            out=mk_c[:],
            in0=ic_c[:],
            in1=lmask_sb[:, 16384:16640],
            op=mybir.AluOpType.mult,
        )
        for r in range(16):
            nc.tensor.matmul(
                psum_v[:, 1024:J],
                swide_sb[:, 120 - 8 * r:248 - 8 * r],
                mk_c[:, 16 * r:16 * (r + 1)],
                start=(r == 0),
                stop=(r == 15),
            )

        # --- cum = scan(V) + cross-partition offsets ------------------------
        cum_part = const.tile([P, J], f32)
        nc.vector.tensor_tensor_scan(
            out=cum_part[:],
            data0=psum_v[:],
            data1=zeros_j[:],
            initial=0.0,
            op0=mybir.AluOpType.add,
            op1=mybir.AluOpType.add,
        )
        pt_ps = psm.tile([P, 1], f32, tag="pt")
        nc.tensor.matmul(
            pt_ps[:], triex_sb[:], cum_part[:, J - 1:J], start=True, stop=True
        )
        cum_sb = const.tile([P, J], f32)
        nc.vector.tensor_scalar(
            out=cum_sb[:], in0=cum_part[:], scalar1=pt_ps[:], scalar2=None,
            op0=mybir.AluOpType.add,
        )
        nc.sync.dma_start(
            cum_dram[:].rearrange("(p j) one -> p (j one)", p=P), cum_sb[:]
        )

        # --- segment ends: per-k pipelined gather -> shift -> sv -> out ------
        g1s = const.tile([P, K16 + 1], f32)
        nc.vector.memset(g1s[:, 0:1], 0.0)
        out_sb = const.tile([P, K16, NL], f32)
        out_view = out[:].rearrange("(k p) l -> p k l", p=P)
        sv = const.tile([P, K16], f32)
        for k in range(K16):
            gt = ge.tile([P, 1], f32, tag="gt")
            nc.gpsimd.indirect_dma_start(
                out=gt[:],
                out_offset=None,
                in_=cum_dram[:],
                in_offset=IndirectOffsetOnAxis(ap=endoff_sb[:, k:k + 1], axis=0),
            )
            nc.scalar.copy(g1s[:, 1 + k:2 + k], gt[:])
            g2k = psm.tile([P, 1], f32, tag="g2k")
            nc.tensor.matmul(
                g2k[:], subm_sb[:], g1s[:, 1 + k:2 + k], start=True, stop=False
            )
            nc.tensor.matmul(
                g2k[:], cornm_sb[:], g1s[:, k:1 + k], start=False, stop=True
            )
            nc.vector.tensor_tensor(
                out=sv[:, k:k + 1], in0=g1s[:, 1 + k:2 + k], in1=g2k[:],
                op=mybir.AluOpType.subtract,
            )
            nc.vector.tensor_tensor(
                out=sv[:, k:k + 1], in0=sv[:, k:k + 1], in1=rec[:, k:k + 1],
                op=mybir.AluOpType.mult,
            )
            nc.vector.tensor_scalar(
                out=out_sb[:, k, :], in0=w_b[:],
                scalar1=sv[:, k:k + 1], scalar2=None,
                op0=mybir.AluOpType.mult,
            )
            nc.vector.tensor_tensor(
                out=out_sb[:, k, :], in0=out_sb[:, k, :], in1=b_b[:],
                op=mybir.AluOpType.add,
            )
            if k % 4 == 3:
                nc.sync.dma_start(
                    out_view[0:P, k - 3:k + 1, :], out_sb[:, k - 3:k + 1, :]
                )

    _split_waits(nc)
    return nc


_PROGRAM = None


def _get_program():
    global _PROGRAM
    if _PROGRAM is None:
        _PROGRAM = build_program()
    return _PROGRAM


def _host_prep(tokens, segment_ids, word_vectors, hidden_w, hidden_b):
    """Integer-only preprocessing: shard + slot/lane/end index tensors."""
    tokens = np.asarray(tokens)
    segment_ids = np.asarray(segment_ids)
    wv = np.asarray(word_vectors, dtype=np.float32)
    hw = np.asarray(hidden_w, dtype=np.float32)
    hbv = np.asarray(hidden_b, dtype=np.float32)

    wv_pad = np.zeros((VOC_PAD, D), dtype=np.float32)
    wv_pad[:VOC] = wv
    hwT = np.ascontiguousarray(hw.T)
    hbr = hbv.reshape(1, NL)

    triex = np.triu(np.ones((P, P), dtype=np.float32), 1)       # pi < po
    subm = np.zeros((P, P), dtype=np.float32)
    subm[np.arange(P - 1), np.arange(1, P)] = 1.0               # po = pi+1
    cornm = np.zeros((P, P), dtype=np.float32)
    cornm[P - 1, 0] = 1.0                                       # po=0 <- pi=127
    swide = np.zeros((P, 248), dtype=np.float32)
    swide[np.arange(P), 120 + np.arange(P) // 16] = 1.0
    swide = swide.astype(ml_dtypes.bfloat16)

    cuts = np.searchsorted(segment_ids, np.arange(NCORES + 1) * SENT_PER_CORE)
    in_maps = []
    for c in range(NCORES):
        lo, hi = int(cuts[c]), int(cuts[c + 1])
        n = hi - lo
        assert n <= SLOTS, f"shard {c}: {n} > {SLOTS}"
        toks = tokens[lo:hi].astype(np.int64)
        segs = (segment_ids[lo:hi] - c * SENT_PER_CORE).astype(np.int64)

        q = np.arange(n)
        p = q // J                 # partition-major token layout
        j = q % J
        g = p % 8
        r = p // 8
        i = r * J + j
        lane = toks // LANE
        e = toks % LANE

        icidx_a = np.zeros((P, 1040), dtype=np.uint16)
        lo_j = j < 1024
        icidx_a[16 * g[lo_j] + j[lo_j] % 16,
                64 * r[lo_j] + j[lo_j] // 16] = e[lo_j].astype(np.uint16)
        hi_j = ~lo_j
        it = r[hi_j] * 16 + (j[hi_j] - 1024)       # tail slot index [0, 256)
        icidx_a[16 * g[hi_j] + it % 16, 1024 + it // 16] = e[hi_j].astype(np.uint16)
        lmask_a = np.zeros((P, IC), dtype=np.float32)
        im = np.where(j < 1024, 1024 * r + j, 16384 + r * 16 + (j - 1024))
        lmask_a[16 * g + lane, im] = 1.0
        lmask_a = lmask_a.astype(ml_dtypes.bfloat16)

        ends_q = np.searchsorted(segs, np.arange(SENT_PER_CORE), side="right") - 1
        lens_a = np.bincount(segs, minlength=SENT_PER_CORE).astype(np.float32)
        assert lens_a.min() >= 1, f"shard {c} has empty segments"
        endoff_a = ends_q.reshape(K16, P).T.astype(np.int32).copy()
        lens_t = lens_a.reshape(K16, P).T.astype(np.float32).copy()

        in_maps.append(
            {
                "wvs": wv_pad[c * VSHARD:(c + 1) * VSHARD],
                "icidx": icidx_a,
                "lmask": lmask_a,
                "endoff": endoff_a,
                "lens": lens_t,
                "swide": swide,
                "triex": triex,
                "subm": subm,
                "cornm": cornm,
                "hwT": hwT,
                "hb": hbr,
            }
        )
    return in_maps


def kernel(tokens, segment_ids, word_vectors, hidden_w, hidden_b):
    from concourse.bass_utils import run_bass_kernel_spmd

    in_maps = _host_prep(tokens, segment_ids, word_vectors, hidden_w, hidden_b)
    nc = _get_program()
    res = run_bass_kernel_spmd(nc, in_maps, list(range(NCORES)))
    return np.concatenate([res.results[c]["out"] for c in range(NCORES)], axis=0)


# revision 8
# speedup vs baseline: 1.2265x; 1.0055x over previous
"""Trainium2 Bass kernel for nn_Net_28544352649361 (segment_reduce), v2.

Per core (2048 sentences, <=133120 tokens laid out partition-major:
token q -> (p = q // 1040, j = q % 1040)):
  1. rowsum[v] = sum_d wv[v, d]: reduce own 12544-row vocab shard (6.4MB
     dense read + DVE reduce), AllGather -> full [100352] table in DRAM.
  2. Lane tables: SBUF data[p, e] = rowsum[6272*(p%16) + e] via one
     broadcast-AP DMA; each 16-partition GPSIMD group holds the whole vocab.
  3. Gather: 8x indirect_copy fetches entry e(v)=v%6272 on all 16 lanes of
     the owning group; host bf16... f32 mask keeps lane v//6272; bf16
     mask-product collapses via 0/1-stationary matmuls that route slot
     (g, r*1040+j) -> psum_v[8r+g, j] = V in token order.
  4. cum: DVE tensor_tensor_scan along free dim + cross-partition exclusive
     prefix (tri matmul) + per-partition offset add; cum -> DRAM (flat = q).
  5. Segment sums: 16 indirect gathers of cum at segment-end positions;
     predecessor ends via shift matmuls; sv = (G1 - G2) / max(len, 1).
  6. out[s, l] = sv[s] * rowsum(hidden_w)[l] + hb[l].
Host prep is integer-only: shard cuts, slot/lane indices, end offsets, lens.
"""

import sys

sys.path.insert(0, "/opt/trn_rl_repo")

from contextlib import ExitStack

import dataclasses
import numpy as np
import ml_dtypes

import concourse.bass as bass
import concourse.tile as tile
from concourse import mybir
from concourse.bass import IndirectOffsetOnAxis
from concourse.vector_clock import ScopedClock

P = 128
J = 1040                  # free-dim tokens per partition; slots = 128*J
SLOTS = P * J             # 133120 >= max shard (131371)
IC = 16 * J               # indirect_copy output columns (8 groups x 2 rounds)
NCHUNK = 8
CCOL = IC // NCHUNK       # 2080 ic-columns per chunk (= 2 rounds of J)
D = 128
NL = 128
NSENT = 16384
NCORES = 8
SENT_PER_CORE = NSENT // NCORES   # 2048
K16 = SENT_PER_CORE // P          # 16
VOC = 100000
VOC_PAD = 100352
VSHARD = VOC_PAD // NCORES        # 12544
LANE = VOC_PAD // 16              # 6272
U = VSHARD // P                   # 98 rows per partition in rowsum build

_num_splits = [0]


# ---------------------------------------------------------------------------
# Workarounds for this walrus build (accepts only ONE sync-wait per
# instruction) and Tile's drain path.
# ---------------------------------------------------------------------------
def _split_drain_and_barrier(self, tick_clock, wait_clock):
    nc = self.nc
    drain_inst = nc.sync.drain()
    wait_clock.add_sem_waits(
        drain_inst.ins, ScopedClock({None: tick_clock.global_clock})
    )
    mi = drain_inst.ins
    si = mi.sync_info
    if si is not None and si.on_wait is not None and len(si.on_wait) > 1:
        waits = list(si.on_wait)
        si.on_wait = waits[:1]
        for w in waits[1:]:
            extra = nc.sync.drain()
            emi = extra.ins
            esi = emi.sync_info
            if esi is None:
                emi.sync_info = mybir.SyncInfo(on_wait=[w], on_update=[])
            else:
                esi.on_wait = [w]
    nc.all_engine_barrier()
    assert self.sems is not None
    popped = nc._tile_sem_poison_stack.pop()
    assert popped is self._sem_poison
    nc.clear_and_free_semaphores(list(self.sems.allocated().values()))
    nc.all_engine_barrier()


def _apply_patches():
    if getattr(tile, "_segred_patched", False):
        return
    tile.TileContext._drain_and_barrier = _split_drain_and_barrier
    tile._segred_patched = True


def _split_waits(nc):
    """Hoist surplus sync-waits onto same-engine NoOps placed just before the
    waiter; the engine sequencer executes them in order."""
    import bass_rust

    for f in nc.m.functions:
        for bb in f.blocks:
            new_list = []
            changed = False
            for inst in bb.instructions:
                si = inst.sync_info
                if si is not None and si.on_wait is not None and len(si.on_wait) > 1:
                    waits = list(si.on_wait)
                    si.on_wait = waits[-1:]
                    for w in waits[:-1]:
                        _num_splits[0] += 1
                        nop = bass_rust.InstNoOp(
                            name=f"WSPLIT-{_num_splits[0]}", ins=[], outs=[]
                        )
                        nop.engine = inst.engine
                        nop.sync_info = mybir.SyncInfo(on_wait=[w], on_update=[])
                        new_list.append(nop)
                    changed = True
                new_list.append(inst)
            if changed:
                bb.instructions = new_list


# ---------------------------------------------------------------------------
# Device program (identical for all cores; per-core data via in_maps)
# ---------------------------------------------------------------------------
def build_program():
    _apply_patches()
    nc = bass.Bass(num_devices=NCORES)
    f32 = mybir.dt.float32
    i32 = mybir.dt.int32
    u16 = mybir.dt.uint16
    bf16 = mybir.dt.bfloat16

    wvT = nc.declare_dram_parameter("wvT", [D, VSHARD], bf16, isOutput=False)
    icidx = nc.declare_dram_parameter("icidx", [P, 1040], u16, isOutput=False)
    lmask = nc.declare_dram_parameter("lmask", [P, IC], bf16, isOutput=False)
    endoff = nc.declare_dram_parameter("endoff", [P, K16], i32, isOutput=False)
    lens = nc.declare_dram_parameter("lens", [P, K16], f32, isOutput=False)
    swide = nc.declare_dram_parameter("swide", [P, 248], bf16, isOutput=False)
    triex = nc.declare_dram_parameter("triex", [P, P], f32, isOutput=False)
    subm = nc.declare_dram_parameter("subm", [P, P], f32, isOutput=False)
    cornm = nc.declare_dram_parameter("cornm", [P, P], f32, isOutput=False)
    hwT = nc.declare_dram_parameter("hwT", [D, NL], f32, isOutput=False)
    hb = nc.declare_dram_parameter("hb", [1, NL], f32, isOutput=False)
    out = nc.declare_dram_parameter("out", [SENT_PER_CORE, NL], f32, isOutput=True)

    rs_shard = nc.dram_tensor("rs_shard", [1, VSHARD], bf16)
    rs_full = nc.dram_tensor("rs_full", [NCORES, VSHARD], bf16)
    cum_dram = nc.dram_tensor("cum_dram", [SLOTS, 1], f32)

    with ExitStack() as ctx:
        tc = ctx.enter_context(tile.TileContext(nc))
        const = ctx.enter_context(tc.tile_pool(name="const", bufs=1))
        gp = ctx.enter_context(tc.tile_pool(name="g", bufs=3))
        mp = ctx.enter_context(tc.tile_pool(name="m", bufs=2))
        small = ctx.enter_context(tc.tile_pool(name="small", bufs=3))
        ge = ctx.enter_context(tc.tile_pool(name="ge", bufs=16))
        pbig = ctx.enter_context(tc.tile_pool(name="pb", bufs=1, space="PSUM"))
        psm = ctx.enter_context(tc.tile_pool(name="psm", bufs=1, space="PSUM"))

        # --- rowsum shard first: PE reduce over transposed bf16 wv ----------
        # wvT[d, row]: rowsum = ones^T wvT, 512-col matmul chunks -> psum
        # [1, 512] -> alternate ACT/DVE copies into a flat [1, 12544] row.
        ones_pb = const.tile([P, 1], bf16)
        nc.vector.memset(ones_pb[:], 1.0)
        rs_row = const.tile([1, VSHARD], bf16)
        wt_bounds = [(2048 * t, min(2048 * (t + 1), VSHARD)) for t in range(7)]
        wt_tiles = []
        for t, (c0, c1) in enumerate(wt_bounds):
            wt = gp.tile([P, c1 - c0], bf16, tag=f"wvT{t % 3}")
            nc.sync.dma_start(wt[:], wvT[:, c0:c1])
            wt_tiles.append(wt)
        flip = 0
        for t, (c0, c1) in enumerate(wt_bounds):
            for s0 in range(c0, c1, 512):
                s1 = min(s0 + 512, c1)
                pr = psm.tile([1, s1 - s0], f32, tag=f"pr{flip % 2}")
                nc.tensor.matmul(
                    pr[:], ones_pb[:], wt_tiles[t][:, s0 - c0:s1 - c0],
                    start=True, stop=True,
                )
                if flip % 2 == 0:
                    nc.scalar.copy(rs_row[:, s0:s1], pr[:])
                else:
                    nc.vector.tensor_scalar(
                        out=rs_row[:, s0:s1], in0=pr[:],
                        scalar1=0.0, scalar2=None,
                        op0=mybir.AluOpType.add,
                    )
                flip += 1
        nc.sync.dma_start(rs_shard[:], rs_row[:])
        nc.gpsimd.collective_compute(
            "AllGather",
            mybir.AluOpType.bypass,
            replica_groups=[list(range(NCORES))],
            ins=[rs_shard[:].opt()],
            outs=[rs_full[:].opt()],
        )

        # --- constants / small inputs (overlap the AllGather window) --------
        icidx_sb = const.tile([P, 1040], u16)
        nc.sync.dma_start(icidx_sb[:], icidx[:])
        lmask_sb = const.tile([P, IC], bf16)
        for q0 in range(0, IC, IC // 4):
            nc.sync.dma_start(
                lmask_sb[:, q0:q0 + IC // 4], lmask[:, q0:q0 + IC // 4]
            )
        endoff_sb = const.tile([P, K16], i32)
        nc.sync.dma_start(endoff_sb[:], endoff[:])
        lens_sb = const.tile([P, K16], f32)
        nc.sync.dma_start(lens_sb[:], lens[:])
        swide_sb = const.tile([P, 248], bf16)
        nc.sync.dma_start(swide_sb[:], swide[:])
        triex_sb = const.tile([P, P], f32)
        nc.sync.dma_start(triex_sb[:], triex[:])
        subm_sb = const.tile([P, P], f32)
        nc.sync.dma_start(subm_sb[:], subm[:])
        cornm_sb = const.tile([P, P], f32)
        nc.sync.dma_start(cornm_sb[:], cornm[:])
        hwT_sb = const.tile([D, NL], f32)
        nc.sync.dma_start(hwT_sb[:], hwT[:])
        hb_sb = const.tile([1, NL], f32)
        nc.sync.dma_start(hb_sb[:], hb[:])
        ones_p = const.tile([P, 1], f32)
        nc.vector.memset(ones_p[:], 1.0)
        ones_1 = const.tile([1, P], f32)
        nc.vector.memset(ones_1[:], 1.0)
        zeros_j = const.tile([P, J], f32)
        nc.vector.memset(zeros_j[:], 0.0)

        # --- w_b / b_b broadcast --------------------------------------------
        wrow_ps = psm.tile([1, NL], f32, tag="ps1")
        nc.tensor.matmul(wrow_ps[:], ones_p[:], hwT_sb[:], start=True, stop=True)
        wrow = small.tile([1, NL], f32)
        nc.scalar.copy(wrow[:], wrow_ps[:])
        wb_ps = psm.tile([P, NL], f32, tag="ps2")
        nc.tensor.matmul(wb_ps[:], ones_1[:], wrow[:], start=True, stop=True)
        w_b = const.tile([P, NL], f32)
        nc.scalar.copy(w_b[:], wb_ps[:])
        bb_ps = psm.tile([P, NL], f32, tag="ps2")
        nc.tensor.matmul(bb_ps[:], ones_1[:], hb_sb[:], start=True, stop=True)
        b_b = const.tile([P, NL], f32)
        nc.scalar.copy(b_b[:], bb_ps[:])
        lm = small.tile([P, K16], f32)
        nc.vector.tensor_scalar_max(lm[:], lens_sb[:], 1.0)
        rec = const.tile([P, K16], f32)
        nc.vector.reciprocal(rec[:], lm[:])

        # --- lane tables: data[p, e] = rowsum[6272*(p%16) + e] --------------
        data_sb = const.tile([P, LANE], bf16)
        lanes16 = rs_full[:].rearrange("c (l e) -> (c l) e", l=2)   # [16, 6272]
        for g in range(8):
            eng = (nc.sync, nc.gpsimd, nc.scalar)[g % 3]
            eng.dma_start(data_sb[16 * g:16 * g + 16, :], lanes16[:, :])

        # --- gather + mask + collapse into psum_v[128, J] -------------------
        # indirect_copy caps at 1024 valid indices -> 1024 + 16 split per round
        psum_v = pbig.tile([P, 1024], f32, tag="pv")
        psum_vt = pbig.tile([P, J - 1024], f32, tag="pvt")
        for r in range(16):
            ic_a = gp.tile([P, 1024], bf16, tag="ica")
            nc.gpsimd.indirect_copy(
                out=ic_a[:],
                data=data_sb[:],
                idxs=icidx_sb[:, 64 * r:64 * r + 64],
                i_know_ap_gather_is_preferred=True,
            )
            mk_t = mp.tile([P, 1024], bf16, tag="mk")
            nc.vector.tensor_tensor(
                out=mk_t[:],
                in0=ic_a[:],
                in1=lmask_sb[:, 1024 * r:1024 * (r + 1)],
                op=mybir.AluOpType.mult,
            )
            stat = swide_sb[:, 120 - 8 * r:248 - 8 * r]
            for j0, j1 in ((0, 512), (512, 1024)):
                nc.tensor.matmul(
                    psum_v[:, j0:j1],
                    stat,
                    mk_t[:, j0:j1],
                    start=(r == 0),
                    stop=(r == 15),
                )
        # tail: slots j in [1024, 1040) for all 16 rounds in one gather
        ic_c = gp.tile([P, 256], bf16, tag="icc")
        nc.gpsimd.indirect_copy(
            out=ic_c[:],
            data=data_sb[:],
            idxs=icidx_sb[:, 1024:1040],
            i_know_ap_gather_is_preferred=True,
        )
        mk_c = mp.tile([P, 256], bf16, tag="mkc")
        nc.vector.tensor_tensor(
            out=mk_c[:],
            in0=ic_c[:],
            in1=lmask_sb[:, 16384:16640],
            op=mybir.AluOpType.mult,
        )
        for r in range(16):
            nc.tensor.matmul(
                psum_vt[:],
                swide_sb[:, 120 - 8 * r:248 - 8 * r],
                mk_c[:, 16 * r:16 * (r + 1)],
                start=(r == 0),
                stop=(r == 15),
            )

        # --- cum = scan(V) + cross-partition offsets ------------------------
        cum_part = const.tile([P, J], f32)
        nc.vector.tensor_tensor_scan(
            out=cum_part[:, 0:1024],
            data0=psum_v[:],
            data1=zeros_j[:, 0:1024],
            initial=0.0,
            op0=mybir.AluOpType.add,
            op1=mybir.AluOpType.add,
        )
        nc.vector.tensor_tensor_scan(
            out=cum_part[:, 1024:J],
            data0=psum_vt[:],
            data1=zeros_j[:, 1024:J],
            initial=cum_part[:, 1023:1024],
            op0=mybir.AluOpType.add,
            op1=mybir.AluOpType.add,
        )
        pt_ps = psm.tile([P, 1], f32, tag="pt")
        nc.tensor.matmul(
            pt_ps[:], triex_sb[:], cum_part[:, J - 1:J], start=True, stop=True
        )
        cum_sb = const.tile([P, J], f32)
        nc.vector.tensor_scalar(
            out=cum_sb[:], in0=cum_part[:], scalar1=pt_ps[:], scalar2=None,
            op0=mybir.AluOpType.add,
        )
        nc.sync.dma_start(
            cum_dram[:].rearrange("(p j) one -> p (j one)", p=P), cum_sb[:]
        )

        # --- segment ends: per-k pipelined gather -> shift -> sv -> out ------
        g1s = const.tile([P, K16 + 1], f32)
        nc.vector.memset(g1s[:, 0:1], 0.0)
        out_sb = const.tile([P, K16, NL], f32)
        out_view = out[:].rearrange("(k p) l -> p k l", p=P)
        sv = const.tile([P, K16], f32)
        for k in range(K16):
            gt = ge.tile([P, 1], f32, tag="gt")
            nc.gpsimd.indirect_dma_start(
                out=gt[:],
                out_offset=None,
                in_=cum_dram[:],
                in_offset=IndirectOffsetOnAxis(ap=endoff_sb[:, k:k + 1], axis=0),
            )
            nc.scalar.copy(g1s[:, 1 + k:2 + k], gt[:])
            g2k = psm.tile([P, 1], f32, tag="pt")
            nc.tensor.matmul(
                g2k[:], subm_sb[:], g1s[:, 1 + k:2 + k], start=True, stop=False
            )
            nc.tensor.matmul(
                g2k[:], cornm_sb[:], g1s[:, k:1 + k], start=False, stop=True
            )
            nc.vector.tensor_tensor(
                out=sv[:, k:k + 1], in0=g1s[:, 1 + k:2 + k], in1=g2k[:],
                op=mybir.AluOpType.subtract,
            )
            nc.vector.tensor_scalar(
                out=out_sb[:, k, :], in0=w_b[:],
                scalar1=sv[:, k:k + 1], scalar2=rec[:, k:k + 1],
                op0=mybir.AluOpType.mult,
                op1=mybir.AluOpType.mult,
            )
            nc.vector.tensor_tensor(
                out=out_sb[:, k, :], in0=out_sb[:, k, :], in1=b_b[:],
                op=mybir.AluOpType.add,
            )
            if k % 4 == 3:
                nc.sync.dma_start(
                    out_view[0:P, k - 3:k + 1, :], out_sb[:, k - 3:k + 1, :]
                )

    _split_waits(nc)
    return nc


_PROGRAM = None


def _get_program():
    global _PROGRAM
    if _PROGRAM is None:
        _PROGRAM = build_program()
    return _PROGRAM


def _host_prep(tokens, segment_ids, word_vectors, hidden_w, hidden_b):
    """Integer-only preprocessing: shard + slot/lane/end index tensors."""
    tokens = np.asarray(tokens)
    segment_ids = np.asarray(segment_ids)
    wv = np.asarray(word_vectors, dtype=np.float32)
    hw = np.asarray(hidden_w, dtype=np.float32)
    hbv = np.asarray(hidden_b, dtype=np.float32)

    wv_pad = np.zeros((VOC_PAD, D), dtype=np.float32)
    wv_pad[:VOC] = wv
    wvT_bf = np.ascontiguousarray(wv_pad.T).astype(ml_dtypes.bfloat16)
    hwT = np.ascontiguousarray(hw.T)
    hbr = hbv.reshape(1, NL)

    triex = np.triu(np.ones((P, P), dtype=np.float32), 1)       # pi < po
    subm = np.zeros((P, P), dtype=np.float32)
    subm[np.arange(P - 1), np.arange(1, P)] = 1.0               # po = pi+1
    cornm = np.zeros((P, P), dtype=np.float32)
    cornm[P - 1, 0] = 1.0                                       # po=0 <- pi=127
    swide = np.zeros((P, 248), dtype=np.float32)
    swide[np.arange(P), 120 + np.arange(P) // 16] = 1.0
    swide = swide.astype(ml_dtypes.bfloat16)

    cuts = np.searchsorted(segment_ids, np.arange(NCORES + 1) * SENT_PER_CORE)
    in_maps = []
    for c in range(NCORES):
        lo, hi = int(cuts[c]), int(cuts[c + 1])
        n = hi - lo
        assert n <= SLOTS, f"shard {c}: {n} > {SLOTS}"
        toks = tokens[lo:hi].astype(np.int64)
        segs = (segment_ids[lo:hi] - c * SENT_PER_CORE).astype(np.int64)

        q = np.arange(n)
        p = q // J                 # partition-major token layout
        j = q % J
        g = p % 8
        r = p // 8
        i = r * J + j
        lane = toks // LANE
        e = toks % LANE

        icidx_a = np.zeros((P, 1040), dtype=np.uint16)
        lo_j = j < 1024
        icidx_a[16 * g[lo_j] + j[lo_j] % 16,
                64 * r[lo_j] + j[lo_j] // 16] = e[lo_j].astype(np.uint16)
        hi_j = ~lo_j
        it = r[hi_j] * 16 + (j[hi_j] - 1024)       # tail slot index [0, 256)
        icidx_a[16 * g[hi_j] + it % 16, 1024 + it // 16] = e[hi_j].astype(np.uint16)
        lmask_a = np.zeros((P, IC), dtype=np.float32)
        im = np.where(j < 1024, 1024 * r + j, 16384 + r * 16 + (j - 1024))
        lmask_a[16 * g + lane, im] = 1.0
        lmask_a = lmask_a.astype(ml_dtypes.bfloat16)

        ends_q = np.searchsorted(segs, np.arange(SENT_PER_CORE), side="right") - 1
        lens_a = np.bincount(segs, minlength=SENT_PER_CORE).astype(np.float32)
        assert lens_a.min() >= 1, f"shard {c} has empty segments"
        endoff_a = ends_q.reshape(K16, P).T.astype(np.int32).copy()
        lens_t = lens_a.reshape(K16, P).T.astype(np.float32).copy()

        in_maps.append(
            {
                "wvT": np.ascontiguousarray(
                    wvT_bf[:, c * VSHARD:(c + 1) * VSHARD]),
                "icidx": icidx_a,
                "lmask": lmask_a,
                "endoff": endoff_a,
                "lens": lens_t,
                "swide": swide,
                "triex": triex,
                "subm": subm,
                "cornm": cornm,
                "hwT": hwT,
                "hb": hbr,
            }
        )
    return in_maps


def kernel(tokens, segment_ids, word_vectors, hidden_w, hidden_b):
    from concourse.bass_utils import run_bass_kernel_spmd

    in_maps = _host_prep(tokens, segment_ids, word_vectors, hidden_w, hidden_b)
    nc = _get_program()
    res = run_bass_kernel_spmd(nc, in_maps, list(range(NCORES)))
    return np.concatenate([res.results[c]["out"] for c in range(NCORES)], axis=0)
